# revision 21
# baseline (speedup 1.0000x reference)
"""Trainium2 Bass kernel for nn_DecoderAttention (B=32, LQ=256, LK=2048, D=512, H=8).

Data-parallel over batch across 8 NeuronCores (4 batch items each).
All matmuls bf16. v2: software-pipelined emission keeps the PE saturated
and HAM-warm end to end.

Structure per batch b (steady state):
  64 "attention units" (t_ in 0..3 x lc in 0..15): S^T matmul (one
  [128,512] MM per lc via block-diag qpPad), ACT Exp (mask bias per
  l-partition), two U accumulation MMs ([v_e|1] rows 0:65 / [1|v_o] rows
  63:128 so ctx lands lane-aligned).  Between units, "filler" closures
  are consumed: out_proj/final-scores of batch b-1, k2 projection of b,
  and mask/q/qp/k-transpose/kp/vp of batch b+1.  The ACT-bound Exp
  stream thus always overlaps PE-bound projection work.

Algebraic folds (all exact):
  - bk dropped: adding a per-query constant to scores cancels in softmax.
  - bv folded into bo' = bo + Wo@bv.
  - bks folded into a per-q tanh bias: bias_q = ncT^T @ (bks/sqrt(D)).
PSUM: tr(2,bf16) + mm(2) + st(2) + u(2) = 8 banks.  mm rotation is
shared by qp/kp/vp/k2/out_proj/zb/biasq/final-score groups; st by S only.
Z reciprocal via reciprocal_approx_fast (5x faster than DVE reciprocal).
"""
import sys

sys.path.insert(0, "/opt/trn_rl_repo")

from collections import deque

import numpy as np

import concourse.bass as bass
import concourse.bacc as bacc
import concourse.mybir as mybir
import concourse.tile as tile
from concourse import bass_utils
from concourse.masks import make_identity

F32 = mybir.dt.float32
BF16 = mybir.dt.bfloat16
U8 = mybir.dt.uint8
AF = mybir.ActivationFunctionType

B, LQ, LK, D, H = 32, 256, 2048, 512, 8
HD = D // H              # 64
NCORES = 8
BPC = B // NCORES        # 4 batch items per core
NLB = LK // 128          # 16 l-blocks
NG = LK // 512           # 4 l-groups
CLIP = 10.0
FLOAT_MIN = -3.4e38
ISQ_HD = 0.125           # 1/sqrt(64)
ISQ_D = float(1.0 / np.sqrt(512.0))
MASK_BIG = -1e30
W_NAMES = ("Wq", "Wk", "Wv", "Wks", "Wo")
B_OF_W = {"Wq": "bq", "Wk": "bk", "Wv": "bv", "Wo": "bo", "Wks": "bks"}

TRACE = False
LAST_RESULTS = None
_CACHE = {}


def _build(reps=1):
    nc = bacc.Bacc("TRN2", target_bir_lowering=False, debug=False)
    q_d = nc.dram_tensor("q", [BPC, LQ, D], F32, kind="ExternalInput").ap()
    k_d = nc.dram_tensor("k", [BPC, LK, D], F32, kind="ExternalInput").ap()
    m_d = nc.dram_tensor("mask", [BPC, LK], U8, kind="ExternalInput").ap()
    w_d = {n: nc.dram_tensor(n, [D, D], F32, kind="ExternalInput").ap()
           for n in W_NAMES}
    # bk is unused (exact softmax cancellation) - not even declared.
    b_d = {n: nc.dram_tensor(n, [D], F32, kind="ExternalInput").ap()
           for n in ("bq", "bv", "bo", "bks")}
    out_d = nc.dram_tensor("out", [BPC, LQ, LK], F32, kind="ExternalOutput").ap()

    bis = [b for _ in range(reps) for b in range(BPC)]
    NB = len(bis)

    lowp = nc.allow_low_precision("bf16 matmul operands by design")
    lowp.__enter__()
    with tile.TileContext(nc) as tc:
        with (
            tc.tile_pool(name="c1", bufs=1) as c1,          # persistent consts
            tc.tile_pool(name="pb", bufs=2) as pb,          # per-batch persistents
            tc.tile_pool(name="vpap", bufs=32) as vpap,     # vp [v_e|1|1|v_o] tiles
            tc.tile_pool(name="knp", bufs=12) as knp,       # k/q/w natural staging
            tc.tile_pool(name="ktp", bufs=8) as ktp,        # kT group tiles
            tc.tile_pool(name="etp", bufs=4) as etp,        # exp output tiles
            tc.tile_pool(name="thp", bufs=3) as thp,        # final output staging
            tc.tile_pool(name="smal", bufs=2) as smal,      # small working tiles
            tc.tile_pool(name="tr_ps", bufs=1, space="PSUM") as tr_ps,
            tc.tile_pool(name="mm_ps", bufs=2, space="PSUM") as mm_ps,
            tc.tile_pool(name="st_ps", bufs=3, space="PSUM") as st_ps,
            tc.tile_pool(name="u_ps", bufs=2, space="PSUM") as u_ps,
        ):
            # ---------------- one-time setup ----------------
            # HAM warm-up first thing, while everything else boots
            wscr = c1.tile([128, 512], BF16, tag="wscr", name="wscr")
            nc.vector.memset(wscr, 1.0)
            for _wi in range(10):
                wp = mm_ps.tile([128, 512], F32, tag="mm", name="mm")
                nc.tensor.matmul(wp, wscr[:, 0:128], wscr, start=True, stop=True)
            identf = c1.tile([128, 128], F32, tag="identf", name="identf")
            make_identity(nc, identf)
            ident = c1.tile([128, 128], BF16, tag="ident", name="ident")
            nc.vector.tensor_copy(ident, identf)
            onr = c1.tile([1, 64], BF16, tag="onr", name="onr")
            nc.vector.memset(onr, 1.0)
            fmin = c1.tile([128, 1], F32, tag="fmin", name="fmin")
            nc.vector.memset(fmin, FLOAT_MIN)

            # bias columns
            bcol = {}
            for bn in ("bq", "bo", "bks", "bv"):
                ap3 = b_d[bn].rearrange("(c p one) -> c p one", p=128, one=1)
                for c in range(4):
                    t = c1.tile([128, 1], F32, tag=f"b_{bn}_{c}", name=f"b_{bn}_{c}")
                    nc.sync.dma_start(out=t, in_=ap3[c])
                    bcol[(bn, c)] = t
            # bks scaled by 1/sqrt(D), bf16 (for the biasq matmul)
            bksq = c1.tile([128, 4], BF16, tag="bksq", name="bksq")
            for c in range(4):
                nc.vector.tensor_scalar_mul(bksq[:, c:c + 1],
                                            bcol[("bks", c)], ISQ_D)
            # bv bf16 cols (for Wo@bv matmul)
            bvb = c1.tile([128, 4], BF16, tag="bvb", name="bvb")
            for c in range(4):
                nc.vector.tensor_copy(bvb[:, c:c + 1], bcol[("bv", c)])

            # transposed weights WT[(wn, dk)] = [128(din chunk), 512(dout)] bf16
            WT = {}
            drain_flip = [0]

            def drain_copy(dst, src):
                # alternate PSUM->SBUF drains between DVE and ACT
                if drain_flip[0] % 2 == 0:
                    nc.vector.tensor_copy(dst, src)
                else:
                    nc.scalar.activation(dst, src, AF.Copy)
                drain_flip[0] += 1

            for wn in ("Wk", "Wv", "Wq", "Wks", "Wo"):
                wnat = []
                for nj in range(4):
                    t = knp.tile([128, 512], BF16, tag="knat", name="knat")
                    nc.gpsimd.dma_start(
                        out=t, in_=w_d[wn][nj * 128:(nj + 1) * 128, :])
                    wnat.append(t)
                for dk in range(4):
                    pt = tr_ps.tile([128, 512], BF16, tag="tr", name="tr")
                    for nj in range(4):
                        nc.tensor.transpose(
                            pt[:, nj * 128:(nj + 1) * 128],
                            wnat[nj][:, dk * 128:(dk + 1) * 128], ident)
                    wt = c1.tile([128, 512], BF16, tag=f"wt_{wn}_{dk}",
                                 name=f"wt_{wn}_{dk}")
                    drain_copy(wt, pt)
                    WT[(wn, dk)] = wt
                if wn == "Wo":
                    # per-head WoTh[h] = [64(din in head), 512(dout)], base 0
                    WoTh = []
                    for h in range(H):
                        pt = tr_ps.tile([128, 512], BF16, tag="tr", name="tr")
                        for nj in range(4):
                            nc.tensor.transpose(
                                pt[0:64, nj * 128:(nj + 1) * 128],
                                wnat[nj][:, h * 64:(h + 1) * 64], ident)
                        wt = c1.tile([64, 512], BF16, tag=f"woth{h}",
                                     name=f"woth{h}")
                        drain_copy(wt, pt[0:64, :])
                        WoTh.append(wt)

            # bo' = bo + Wo@bv  (per-partition f32 cols)
            pt = mm_ps.tile([128, 512], F32, tag="mm", name="mm")
            for dk in range(4):
                nc.tensor.matmul(pt[0:1, :], bvb[:, dk:dk + 1], WT[("Wo", dk)],
                                 start=(dk == 0), stop=(dk == 3))
            borow = smal.tile([1, 512], BF16, tag="borow", name="borow")
            nc.vector.tensor_copy(borow, pt[0:1, :])
            ptt = tr_ps.tile([128, 512], BF16, tag="tr", name="tr")
            for nj in range(4):
                nc.tensor.transpose(ptt[:, 2 * nj:2 * nj + 1],
                                    borow[0:1, nj * 128:(nj + 1) * 128],
                                    ident[0:1, 0:1])
            wobvT = smal.tile([128, 4], F32, tag="wobvT", name="wobvT")
            for nj in range(4):
                nc.vector.tensor_copy(wobvT[:, nj:nj + 1],
                                      ptt[:, 2 * nj:2 * nj + 1])
            bocol = []
            for nj in range(4):
                t = c1.tile([128, 1], F32, tag=f"bop{nj}", name=f"bop{nj}")
                nc.vector.tensor_scalar_add(out=t, in0=wobvT[:, nj:nj + 1],
                                            scalar1=bcol[("bo", nj)][:, :])
                bocol.append(t)

            # ---------------- per-batch state ----------------
            st_ = {}  # pos -> dict of tiles

            def S(pos):
                if pos not in st_:
                    st_[pos] = {}
                return st_[pos]

            # -------- filler emitters (each returns list of closures) -----
            def f_pre(pos):
                """mask, q, qp, k DMA/transpose, kp, vp for batch at pos."""
                bi = bis[pos]
                s = S(pos)
                cl = []

                def c_mask_dma():
                    m_row = m_d[bi]
                    bcast = bass.AP(tensor=m_row.tensor, offset=m_row.offset,
                                    ap=[[0, 128]] + m_row.ap)
                    s["masku8"] = pb.tile([128, LK], U8, tag="masku8",
                                          name="masku8")
                    nc.gpsimd.dma_start(out=s["masku8"], in_=bcast)
                    s["m16"] = smal.tile([16, 128], U8, tag="m16", name="m16")
                    nc.sync.dma_start(
                        out=s["m16"], in_=m_row.rearrange("(c p) -> c p", c=16))
                cl.append(c_mask_dma)

                def c_mask_tr():
                    m16f = smal.tile([16, 128], BF16, tag="m16f", name="m16f")
                    nc.vector.tensor_copy(m16f, s["m16"])
                    mpt = tr_ps.tile([128, 512], BF16, tag="tr", name="tr")
                    nc.tensor.transpose(mpt[:, 0:16], m16f, ident[0:16, 0:16])
                    s["mb"] = pb.tile([128, 16], F32, tag="mb", name="mb")
                    nc.vector.tensor_scalar_mul(s["mb"], mpt[:, 0:16], MASK_BIG)
                cl.append(c_mask_tr)

                def c_q_dma():
                    s["qnat"] = []
                    for mi in range(2):
                        t = knp.tile([128, 512], BF16, tag="knat", name="knat")
                        nc.gpsimd.dma_start(
                            out=t, in_=q_d[bi, mi * 128:(mi + 1) * 128, :])
                        s["qnat"].append(t)
                cl.append(c_q_dma)

                def c_q_tr(dk):
                    if "qTr" not in s:
                        s["qTr"] = [None] * 4
                    pt = tr_ps.tile([128, 512], BF16, tag="tr", name="tr")
                    for mi in range(2):
                        nc.tensor.transpose(
                            pt[:, mi * 128:(mi + 1) * 128],
                            s["qnat"][mi][:, dk * 128:(dk + 1) * 128], ident)
                    t = pb.tile([128, 256], BF16, tag=f"qtr{dk}", name=f"qtr{dk}")
                    nc.vector.tensor_copy(t, pt[:, 0:256])
                    s["qTr"][dk] = t
                for dk in range(4):
                    cl.append(lambda dk=dk: c_q_tr(dk))

                def c_qp(nj):
                    if "qpPad" not in s:
                        s["qpPad"] = [None] * 4
                    pt = mm_ps.tile([128, 512], F32, tag="mm", name="mm")
                    for dk in range(4):
                        nc.tensor.matmul(
                            pt[:, 0:256],
                            WT[("Wq", dk)][:, nj * 128:(nj + 1) * 128],
                            s["qTr"][dk], start=(dk == 0), stop=(dk == 3))
                    t = pb.tile([128, 512], BF16, tag=f"qpd{nj}", name=f"qpd{nj}")
                    nc.vector.memset(t, 0.0)
                    nc.vector.tensor_scalar_add(
                        out=t[0:64, 0:256], in0=pt[0:64, 0:256],
                        scalar1=bcol[("bq", nj)][0:64, :])
                    nc.vector.tensor_scalar_add(
                        out=t[64:128, 256:512], in0=pt[64:128, 0:256],
                        scalar1=bcol[("bq", nj)][64:128, :])
                    s["qpPad"][nj] = t
                for nj in range(4):
                    cl.append(lambda nj=nj: c_qp(nj))

                s["kpTr"] = [None] * 4
                s["k2Tr"] = [None] * 4
                s["vpa"] = [None] * NLB
                s["kTg"] = {}
                s["knat"] = {}

                def c_k_dma(g):
                    knat = []
                    lbase = g * 512
                    for li in range(4):
                        t = knp.tile([128, 512], BF16, tag="knat", name="knat")
                        nc.gpsimd.dma_start(
                            out=t,
                            in_=k_d[bi, lbase + li * 128:lbase + (li + 1) * 128, :])
                        knat.append(t)
                    s["knat"][g] = knat

                def c_ktr(g, dk):
                    pt = tr_ps.tile([128, 512], BF16, tag="tr", name="tr")
                    for li in range(4):
                        nc.tensor.transpose(
                            pt[:, li * 128:(li + 1) * 128],
                            s["knat"][g][li][:, dk * 128:(dk + 1) * 128], ident)
                    t = ktp.tile([128, 512], BF16, tag="ktg", name="ktg")
                    nc.vector.tensor_copy(t, pt)
                    s["kTg"][(g, dk)] = t

                def c_kp(g, nj):
                    if s["kpTr"][nj] is None:
                        s["kpTr"][nj] = pb.tile([128, LK], BF16, tag=f"kpt{nj}",
                                                name=f"kpt{nj}")
                    pt = mm_ps.tile([128, 512], F32, tag="mm", name="mm")
                    for dk in range(4):
                        nc.tensor.matmul(
                            pt, WT[("Wk", dk)][:, nj * 128:(nj + 1) * 128],
                            s["kTg"][(g, dk)], start=(dk == 0), stop=(dk == 3))
                    nc.vector.tensor_copy(
                        s["kpTr"][nj][:, g * 512:(g + 1) * 512], pt)

                def c_vp(g, lb):
                    pt = mm_ps.tile([128, 512], F32, tag="mm", name="mm")
                    for dk in range(4):
                        nc.tensor.matmul(
                            pt, s["kTg"][(g, dk)][:, lb * 128:(lb + 1) * 128],
                            WT[("Wv", dk)], start=(dk == 0), stop=(dk == 3))
                    vt = vpap.tile([128, 520], BF16, tag="vpa", name="vpa")
                    vt3 = vt.rearrange("p (h c) -> p h c", c=65)
                    nc.vector.tensor_copy(
                        vt3[:, :, 0:64],
                        pt.rearrange("p (h c) -> p h c", c=64))
                    nc.vector.memset(vt3[:, :, 64:65], 1.0)
                    s["vpa"][g * 4 + lb] = vt

                def c_k2(g, nj):
                    if s["k2Tr"][nj] is None:
                        s["k2Tr"][nj] = pb.tile([128, LK], BF16, tag=f"k2t{nj}",
                                                name=f"k2t{nj}")
                    pt = mm_ps.tile([128, 512], F32, tag="mm", name="mm")
                    for dk in range(4):
                        nc.tensor.matmul(
                            pt, WT[("Wks", dk)][:, nj * 128:(nj + 1) * 128],
                            s["kTg"][(g, dk)], start=(dk == 0), stop=(dk == 3))
                    nc.scalar.activation(
                        s["k2Tr"][nj][:, g * 512:(g + 1) * 512], pt, AF.Copy)

                # k DMAs for the first two groups go to the very front so
                # the gpsimd queue starts them a full round early
                cl.insert(0, lambda: c_k_dma(0))
                cl.insert(2, lambda: c_k_dma(1))
                for g in range(NG):
                    if g + 2 < NG:
                        cl.append(lambda g=g: c_k_dma(g + 2))
                    for dk in range(4):
                        cl.append(lambda g=g, dk=dk: c_ktr(g, dk))
                    for nj in range(4):
                        cl.append(lambda g=g, nj=nj: c_kp(g, nj))
                    for lb in range(4):
                        cl.append(lambda g=g, lb=lb: c_vp(g, lb))
                    for nj in range(4):
                        cl.append(lambda g=g, nj=nj: c_k2(g, nj))
                return cl

            def f_out(pos):
                """out_proj, biasq, final scores for batch at pos."""
                bi = bis[pos]
                s = S(pos)
                cl = []

                def c_op(nj):
                    if "ncTr" not in s:
                        s["ncTr"] = [None] * 4
                    pt = mm_ps.tile([128, 512], F32, tag="mm", name="mm")
                    for t_ in range(4):
                        for hh in range(2):
                            nc.tensor.matmul(
                                pt[:, 0:256],
                                WoTh[2 * t_ + hh][:, nj * 128:(nj + 1) * 128],
                                s["ctx"][t_][:, hh * 256:(hh + 1) * 256],
                                start=(t_ == 0 and hh == 0),
                                stop=(t_ == 3 and hh == 1))
                    t = pb.tile([128, 256], BF16, tag=f"nct{nj}", name=f"nct{nj}")
                    nc.vector.tensor_scalar_add(out=t, in0=pt[:, 0:256],
                                                scalar1=bocol[nj][:, :])
                    s["ncTr"][nj] = t
                for nj in range(4):
                    cl.append(lambda nj=nj: c_op(nj))

                def c_biasq():
                    pt = mm_ps.tile([128, 512], F32, tag="mm", name="mm")
                    for nk in range(4):
                        nc.tensor.matmul(pt[0:1, 0:256], bksq[:, nk:nk + 1],
                                         s["ncTr"][nk],
                                         start=(nk == 0), stop=(nk == 3))
                    bqrow = smal.tile([1, 256], BF16, tag="bqrow", name="bqrow")
                    nc.vector.tensor_copy(bqrow, pt[0:1, 0:256])
                    ptt = tr_ps.tile([128, 512], BF16, tag="tr", name="tr")
                    for mi in range(2):
                        nc.tensor.transpose(ptt[:, 2 * mi:2 * mi + 1],
                                            bqrow[0:1, mi * 128:(mi + 1) * 128],
                                            ident[0:1, 0:1])
                    s["biasqT"] = pb.tile([128, 2], F32, tag="biasqT",
                                          name="biasqT")
                    for mi in range(2):
                        nc.vector.tensor_copy(s["biasqT"][:, mi:mi + 1],
                                              ptt[:, 2 * mi:2 * mi + 1])
                cl.append(c_biasq)

                def c_fin(mi, lg):
                    pt = mm_ps.tile([128, 512], F32, tag="mm", name="mm")
                    for nk in range(4):
                        nc.tensor.matmul(
                            pt, s["ncTr"][nk][:, mi * 128:(mi + 1) * 128],
                            s["k2Tr"][nk][:, lg * 512:(lg + 1) * 512],
                            start=(nk == 0), stop=(nk == 3))
                    th = thp.tile([128, 512], F32, tag="th", name="th")
                    nc.scalar.activation(th, pt, AF.Tanh,
                                         bias=s["biasqT"][:, mi:mi + 1],
                                         scale=ISQ_D)
                    th2 = thp.tile([128, 512], F32, tag="th2", name="th2")
                    nc.vector.tensor_scalar_mul(th2, th, CLIP)
                    nc.vector.copy_predicated(
                        th2, s["masku8"][:, lg * 512:(lg + 1) * 512],
                        fmin.to_broadcast([128, 512]))
                    nc.sync.dma_start(
                        out=out_d[bi, mi * 128:(mi + 1) * 128,
                                  lg * 512:(lg + 1) * 512],
                        in_=th2)
                for mi in range(2):
                    for lg in range(4):
                        cl.append(lambda mi=mi, lg=lg: c_fin(mi, lg))
                return cl

            # -------- attention unit machinery --------
            def emit_S(pos, t_, lc):
                s = S(pos)
                sp = st_ps.tile([128, 512], F32, tag="st", name="st")
                nc.tensor.matmul(
                    sp, s["kpTr"][t_][:, lc * 128:(lc + 1) * 128],
                    s["qpPad"][t_], start=True, stop=True)
                s[("sp", t_, lc)] = sp

            def emit_exp(pos, t_, lc):
                s = S(pos)
                et = etp.tile([128, 512], BF16, tag="et", name="et")
                nc.scalar.activation(
                    et, s.pop(("sp", t_, lc)), AF.Exp,
                    bias=s["mb"][:, lc:lc + 1], scale=ISQ_HD)
                s[("et", t_, lc)] = et

            def emit_U(pos, t_, lc):
                s = S(pos)
                if lc == 0:
                    s[("u", t_)] = u_ps.tile([128, 512], F32, tag="u", name="u")
                u = s[("u", t_)]
                et = s.pop(("et", t_, lc))
                vt = s["vpa"][lc]
                nc.tensor.matmul(
                    u[0:65, 0:256], vt[:, (2 * t_) * 65:(2 * t_) * 65 + 65],
                    et[:, 0:256], start=(lc == 0), stop=(lc == NLB - 1),
                    skip_group_check=True)
                # start=True on the even head cleared the whole bank; odd
                # head's first matmul relies on has_written=0 -> overwrite.
                nc.tensor.matmul(
                    u[0:65, 256:512],
                    vt[:, (2 * t_ + 1) * 65:(2 * t_ + 1) * 65 + 65],
                    et[:, 256:512], start=False, stop=(lc == NLB - 1),
                    skip_group_check=True)

            def emit_ctx(pos, t_):
                s = S(pos)
                u = s.pop(("u", t_))
                uf = smal.tile([128, 512], F32, tag="uf", name="uf")
                nc.vector.tensor_copy(uf[0:65, :], u[0:65, :])
                zr = smal.tile([1, 512], BF16, tag="zr", name="zr")
                nc.vector.tensor_copy(zr, uf[64:65, :])
                zb = mm_ps.tile([128, 512], F32, tag="mm", name="mm")
                nc.tensor.matmul(zb[0:64, :], onr[0:1, 0:64], zr,
                                 start=True, stop=True)
                rzt = smal.tile([64, 512], F32, tag="rzt", name="rzt")
                nc.vector.reciprocal_approx_fast(rzt, zb[0:64, :])
                if "ctx" not in s:
                    s["ctx"] = [None] * 4
                ct = pb.tile([64, 512], BF16, tag=f"ctx{t_}", name=f"ctx{t_}")
                nc.vector.tensor_mul(ct, uf[0:64, :], rzt)
                s["ctx"][t_] = ct

            def run_units(pos, fillers):
                # 2-deep S lookahead (3 st banks) + 1-deep Exp lookahead:
                # U(i) reads et(i) whose Exp was issued a full unit earlier.
                fill = deque(fillers)
                emit_S(pos, 0, 0)
                emit_S(pos, 0, 1)
                emit_exp(pos, 0, 0)
                for i in range(64):
                    t_, lc = divmod(i, 16)
                    if i + 2 < 64:
                        nt, nl = divmod(i + 2, 16)
                        emit_S(pos, nt, nl)
                    if i + 1 < 64:
                        nt, nl = divmod(i + 1, 16)
                        emit_exp(pos, nt, nl)
                    # fillers BEFORE U: PE chews projection work while ACT
                    # finishes Exp
                    n = -(-len(fill) // (64 - i))  # ceil
                    for _ in range(min(n, 4)):
                        if fill:
                            fill.popleft()()
                    emit_U(pos, t_, lc)
                    if lc == NLB - 1:
                        emit_ctx(pos, t_)
                while fill:
                    fill.popleft()()

            # ---------------- main schedule ----------------
            for clo in f_pre(0):
                clo()
            for pos in range(NB):
                fillers = []
                if pos > 0:
                    fillers += f_out(pos - 1)
                if pos + 1 < NB:
                    fillers += f_pre(pos + 1)
                run_units(pos, fillers)
                # free dead per-batch state
                if pos > 0:
                    st_.pop(pos - 1, None)
            for clo in f_out(NB - 1):
                clo()
    lowp.__exit__(None, None, None)
    nc.finalize()
    return nc


def kernel(**inputs):
    global LAST_RESULTS
    import os
    reps = int(os.environ.get("KERNEL_REPS", "1"))
    key = ("nc", reps)
    if key not in _CACHE:
        _CACHE[key] = _build(reps)
    nc = _CACHE[key]

    q = np.ascontiguousarray(np.asarray(inputs["q"], dtype=np.float32))
    k = np.ascontiguousarray(np.asarray(inputs["k"], dtype=np.float32))
    mask = np.ascontiguousarray(np.asarray(inputs["mask"]).astype(np.uint8))
    ws = {n: np.ascontiguousarray(np.asarray(inputs[n], dtype=np.float32))
          for n in W_NAMES}
    bs = {n: np.ascontiguousarray(np.asarray(inputs[n], dtype=np.float32))
          for n in ("bq", "bv", "bo", "bks")}

    in_maps = []
    for ci in range(NCORES):
        sl = slice(ci * BPC, (ci + 1) * BPC)
        im = {"q": q[sl], "k": k[sl], "mask": mask[sl]}
        im.update(ws)
        im.update(bs)
        in_maps.append(im)

    res = bass_utils.run_bass_kernel_spmd(
        nc, in_maps, core_ids=list(range(NCORES)), trace=TRACE)
    LAST_RESULTS = res
    out = np.concatenate([res.results[ci]["out"] for ci in range(NCORES)], axis=0)
    return out


# revision 23
# speedup vs baseline: 1.0478x; 1.0478x over previous
"""Trainium2 Bass kernel for nn_DecoderAttention (B=32, LQ=256, LK=2048, D=512, H=8).

Data-parallel over batch across 8 NeuronCores (4 batch items each).
All matmuls bf16. v2: software-pipelined emission keeps the PE saturated
and HAM-warm end to end.

Structure per batch b (steady state):
  64 "attention units" (t_ in 0..3 x lc in 0..15): S^T matmul (one
  [128,512] MM per lc via block-diag qpPad), ACT Exp (mask bias per
  l-partition), two U accumulation MMs ([v_e|1] rows 0:65 / [1|v_o] rows
  63:128 so ctx lands lane-aligned).  Between units, "filler" closures
  are consumed: out_proj/final-scores of batch b-1, k2 projection of b,
  and mask/q/qp/k-transpose/kp/vp of batch b+1.  The ACT-bound Exp
  stream thus always overlaps PE-bound projection work.

Algebraic folds (all exact):
  - bk dropped: adding a per-query constant to scores cancels in softmax.
  - bv folded into bo' = bo + Wo@bv.
  - bks folded into a per-q tanh bias: bias_q = ncT^T @ (bks/sqrt(D)).
PSUM: tr(2,bf16) + mm(2) + st(2) + u(2) = 8 banks.  mm rotation is
shared by qp/kp/vp/k2/out_proj/zb/biasq/final-score groups; st by S only.
Z reciprocal via reciprocal_approx_fast (5x faster than DVE reciprocal).
"""
import sys

sys.path.insert(0, "/opt/trn_rl_repo")

from collections import deque

import numpy as np

import concourse.bass as bass
import concourse.bacc as bacc
import concourse.mybir as mybir
import concourse.tile as tile
from concourse import bass_utils
from concourse.masks import make_identity

F32 = mybir.dt.float32
BF16 = mybir.dt.bfloat16
U8 = mybir.dt.uint8
AF = mybir.ActivationFunctionType

B, LQ, LK, D, H = 32, 256, 2048, 512, 8
HD = D // H              # 64
NCORES = 8
BPC = B // NCORES        # 4 batch items per core
NLB = LK // 128          # 16 l-blocks
NG = LK // 512           # 4 l-groups
CLIP = 10.0
FLOAT_MIN = -3.4e38
ISQ_HD = 0.125           # 1/sqrt(64)
ISQ_D = float(1.0 / np.sqrt(512.0))
MASK_BIG = -1e30
W_NAMES = ("Wq", "Wk", "Wv", "Wks", "Wo")
B_OF_W = {"Wq": "bq", "Wk": "bk", "Wv": "bv", "Wo": "bo", "Wks": "bks"}

TRACE = False
LAST_RESULTS = None
_CACHE = {}


def _build(reps=1):
    nc = bacc.Bacc("TRN2", target_bir_lowering=False, debug=False)
    q_d = nc.dram_tensor("q", [BPC, LQ, D], F32, kind="ExternalInput").ap()
    k_d = nc.dram_tensor("k", [BPC, LK, D], F32, kind="ExternalInput").ap()
    m_d = nc.dram_tensor("mask", [BPC, LK], U8, kind="ExternalInput").ap()
    w_d = {n: nc.dram_tensor(n, [D, D], F32, kind="ExternalInput").ap()
           for n in W_NAMES}
    # bk is unused (exact softmax cancellation) - not even declared.
    b_d = {n: nc.dram_tensor(n, [D], F32, kind="ExternalInput").ap()
           for n in ("bq", "bv", "bo", "bks")}
    out_d = nc.dram_tensor("out", [BPC, LQ, LK], F32, kind="ExternalOutput").ap()

    bis = [b for _ in range(reps) for b in range(BPC)]
    NB = len(bis)

    lowp = nc.allow_low_precision("bf16 matmul operands by design")
    lowp.__enter__()
    with tile.TileContext(nc) as tc:
        with (
            tc.tile_pool(name="c1", bufs=1) as c1,          # persistent consts
            tc.tile_pool(name="pb", bufs=2) as pb,          # per-batch persistents
            tc.tile_pool(name="vpap", bufs=32) as vpap,     # vp [v_e|1|1|v_o] tiles
            tc.tile_pool(name="knp", bufs=12) as knp,       # k/q/w natural staging
            tc.tile_pool(name="ktp", bufs=8) as ktp,        # kT group tiles
            tc.tile_pool(name="etp", bufs=4) as etp,        # exp output tiles
            tc.tile_pool(name="thp", bufs=3) as thp,        # final output staging
            tc.tile_pool(name="smal", bufs=2) as smal,      # small working tiles
            tc.tile_pool(name="tr_ps", bufs=2, space="PSUM") as tr_ps,
            tc.tile_pool(name="mm_ps", bufs=2, space="PSUM") as mm_ps,
            tc.tile_pool(name="st_ps", bufs=2, space="PSUM") as st_ps,
            tc.tile_pool(name="u_ps", bufs=2, space="PSUM") as u_ps,
        ):
            # ---------------- one-time setup ----------------
            # HAM warm-up first thing, while everything else boots
            wscr = c1.tile([128, 512], BF16, tag="wscr", name="wscr")
            nc.vector.memset(wscr, 1.0)
            for _wi in range(10):
                wp = mm_ps.tile([128, 512], F32, tag="mm", name="mm")
                nc.tensor.matmul(wp, wscr[:, 0:128], wscr, start=True, stop=True)
            identf = c1.tile([128, 128], F32, tag="identf", name="identf")
            make_identity(nc, identf)
            ident = c1.tile([128, 128], BF16, tag="ident", name="ident")
            nc.vector.tensor_copy(ident, identf)
            onr = c1.tile([1, 64], BF16, tag="onr", name="onr")
            nc.vector.memset(onr, 1.0)
            fmin = c1.tile([128, 1], F32, tag="fmin", name="fmin")
            nc.vector.memset(fmin, FLOAT_MIN)

            # bias columns
            bcol = {}
            for bn in ("bq", "bo", "bks", "bv"):
                ap3 = b_d[bn].rearrange("(c p one) -> c p one", p=128, one=1)
                for c in range(4):
                    t = c1.tile([128, 1], F32, tag=f"b_{bn}_{c}", name=f"b_{bn}_{c}")
                    nc.sync.dma_start(out=t, in_=ap3[c])
                    bcol[(bn, c)] = t
            # bks scaled by 1/sqrt(D), bf16 (for the biasq matmul)
            bksq = c1.tile([128, 4], BF16, tag="bksq", name="bksq")
            for c in range(4):
                nc.vector.tensor_scalar_mul(bksq[:, c:c + 1],
                                            bcol[("bks", c)], ISQ_D)
            # bv bf16 cols (for Wo@bv matmul)
            bvb = c1.tile([128, 4], BF16, tag="bvb", name="bvb")
            for c in range(4):
                nc.vector.tensor_copy(bvb[:, c:c + 1], bcol[("bv", c)])

            # transposed weights WT[(wn, dk)] = [128(din chunk), 512(dout)] bf16
            WT = {}
            drain_flip = [0]

            def drain_copy(dst, src):
                # alternate PSUM->SBUF drains between DVE and ACT
                if drain_flip[0] % 2 == 0:
                    nc.vector.tensor_copy(dst, src)
                else:
                    nc.scalar.activation(dst, src, AF.Copy)
                drain_flip[0] += 1

            for wn in ("Wk", "Wv", "Wq", "Wks", "Wo"):
                wnat = []
                for nj in range(4):
                    t = knp.tile([128, 512], BF16, tag="knat", name="knat")
                    nc.gpsimd.dma_start(
                        out=t, in_=w_d[wn][nj * 128:(nj + 1) * 128, :])
                    wnat.append(t)
                for dk in range(4):
                    pt = tr_ps.tile([128, 512], BF16, tag="tr", name="tr")
                    for nj in range(4):
                        nc.tensor.transpose(
                            pt[:, nj * 128:(nj + 1) * 128],
                            wnat[nj][:, dk * 128:(dk + 1) * 128], ident)
                    wt = c1.tile([128, 512], BF16, tag=f"wt_{wn}_{dk}",
                                 name=f"wt_{wn}_{dk}")
                    drain_copy(wt, pt)
                    WT[(wn, dk)] = wt
                if wn == "Wo":
                    # per-head WoTh[h] = [64(din in head), 512(dout)], base 0
                    WoTh = []
                    for h in range(H):
                        pt = tr_ps.tile([128, 512], BF16, tag="tr", name="tr")
                        for nj in range(4):
                            nc.tensor.transpose(
                                pt[0:64, nj * 128:(nj + 1) * 128],
                                wnat[nj][:, h * 64:(h + 1) * 64], ident)
                        wt = c1.tile([64, 512], BF16, tag=f"woth{h}",
                                     name=f"woth{h}")
                        drain_copy(wt, pt[0:64, :])
                        WoTh.append(wt)

            # bo' = bo + Wo@bv  (per-partition f32 cols)
            pt = mm_ps.tile([128, 512], F32, tag="mm", name="mm")
            for dk in range(4):
                nc.tensor.matmul(pt[0:1, :], bvb[:, dk:dk + 1], WT[("Wo", dk)],
                                 start=(dk == 0), stop=(dk == 3))
            borow = smal.tile([1, 512], BF16, tag="borow", name="borow")
            nc.vector.tensor_copy(borow, pt[0:1, :])
            ptt = tr_ps.tile([128, 512], BF16, tag="tr", name="tr")
            for nj in range(4):
                nc.tensor.transpose(ptt[:, 2 * nj:2 * nj + 1],
                                    borow[0:1, nj * 128:(nj + 1) * 128],
                                    ident[0:1, 0:1])
            wobvT = smal.tile([128, 4], F32, tag="wobvT", name="wobvT")
            for nj in range(4):
                nc.vector.tensor_copy(wobvT[:, nj:nj + 1],
                                      ptt[:, 2 * nj:2 * nj + 1])
            bocol = []
            for nj in range(4):
                t = c1.tile([128, 1], F32, tag=f"bop{nj}", name=f"bop{nj}")
                nc.vector.tensor_scalar_add(out=t, in0=wobvT[:, nj:nj + 1],
                                            scalar1=bcol[("bo", nj)][:, :])
                bocol.append(t)

            # ---------------- per-batch state ----------------
            st_ = {}  # pos -> dict of tiles

            def S(pos):
                if pos not in st_:
                    st_[pos] = {}
                return st_[pos]

            # -------- filler emitters (each returns list of closures) -----
            def f_pre(pos):
                """mask, q, qp, k DMA/transpose, kp, vp for batch at pos."""
                bi = bis[pos]
                s = S(pos)
                cl = []

                def c_mask_dma():
                    m_row = m_d[bi]
                    bcast = bass.AP(tensor=m_row.tensor, offset=m_row.offset,
                                    ap=[[0, 128]] + m_row.ap)
                    s["masku8"] = pb.tile([128, LK], U8, tag="masku8",
                                          name="masku8")
                    nc.gpsimd.dma_start(out=s["masku8"], in_=bcast)
                    s["m16"] = smal.tile([16, 128], U8, tag="m16", name="m16")
                    nc.sync.dma_start(
                        out=s["m16"], in_=m_row.rearrange("(c p) -> c p", c=16))
                cl.append(c_mask_dma)

                def c_mask_tr():
                    m16f = smal.tile([16, 128], BF16, tag="m16f", name="m16f")
                    nc.vector.tensor_copy(m16f, s["m16"])
                    mpt = tr_ps.tile([128, 512], BF16, tag="tr", name="tr")
                    nc.tensor.transpose(mpt[:, 0:16], m16f, ident[0:16, 0:16])
                    s["mb"] = pb.tile([128, 16], F32, tag="mb", name="mb")
                    nc.vector.tensor_scalar_mul(s["mb"], mpt[:, 0:16], MASK_BIG)
                cl.append(c_mask_tr)

                def c_q_dma():
                    s["qnat"] = []
                    for mi in range(2):
                        t = knp.tile([128, 512], BF16, tag="knat", name="knat")
                        nc.gpsimd.dma_start(
                            out=t, in_=q_d[bi, mi * 128:(mi + 1) * 128, :])
                        s["qnat"].append(t)
                cl.append(c_q_dma)

                def c_q_tr(dk):
                    if "qTr" not in s:
                        s["qTr"] = [None] * 4
                    pt = tr_ps.tile([128, 512], BF16, tag="tr", name="tr")
                    for mi in range(2):
                        nc.tensor.transpose(
                            pt[:, mi * 128:(mi + 1) * 128],
                            s["qnat"][mi][:, dk * 128:(dk + 1) * 128], ident)
                    t = pb.tile([128, 256], BF16, tag=f"qtr{dk}", name=f"qtr{dk}")
                    nc.vector.tensor_copy(t, pt[:, 0:256])
                    s["qTr"][dk] = t
                for dk in range(4):
                    cl.append(lambda dk=dk: c_q_tr(dk))

                def c_qp(nj):
                    if "qpPad" not in s:
                        s["qpPad"] = [None] * 4
                    pt = mm_ps.tile([128, 512], F32, tag="mm", name="mm")
                    for dk in range(4):
                        nc.tensor.matmul(
                            pt[:, 0:256],
                            WT[("Wq", dk)][:, nj * 128:(nj + 1) * 128],
                            s["qTr"][dk], start=(dk == 0), stop=(dk == 3))
                    t = pb.tile([128, 512], BF16, tag=f"qpd{nj}", name=f"qpd{nj}")
                    nc.vector.memset(t, 0.0)
                    nc.vector.tensor_scalar_add(
                        out=t[0:64, 0:256], in0=pt[0:64, 0:256],
                        scalar1=bcol[("bq", nj)][0:64, :])
                    nc.vector.tensor_scalar_add(
                        out=t[64:128, 256:512], in0=pt[64:128, 0:256],
                        scalar1=bcol[("bq", nj)][64:128, :])
                    s["qpPad"][nj] = t
                for nj in range(4):
                    cl.append(lambda nj=nj: c_qp(nj))

                s["kpTr"] = [None] * 4
                s["k2Tr"] = [None] * 4
                s["vpa"] = [None] * NLB
                s["kTg"] = {}
                s["knat"] = {}

                def c_k_dma(g):
                    knat = []
                    lbase = g * 512
                    for li in range(4):
                        t = knp.tile([128, 512], BF16, tag="knat", name="knat")
                        nc.gpsimd.dma_start(
                            out=t,
                            in_=k_d[bi, lbase + li * 128:lbase + (li + 1) * 128, :])
                        knat.append(t)
                    s["knat"][g] = knat

                def c_ktr(g, dk):
                    pt = tr_ps.tile([128, 512], BF16, tag="tr", name="tr")
                    for li in range(4):
                        nc.tensor.transpose(
                            pt[:, li * 128:(li + 1) * 128],
                            s["knat"][g][li][:, dk * 128:(dk + 1) * 128], ident)
                    t = ktp.tile([128, 512], BF16, tag="ktg", name="ktg")
                    nc.vector.tensor_copy(t, pt)
                    s["kTg"][(g, dk)] = t

                def c_kp(g, nj):
                    if s["kpTr"][nj] is None:
                        s["kpTr"][nj] = pb.tile([128, LK], BF16, tag=f"kpt{nj}",
                                                name=f"kpt{nj}")
                    pt = mm_ps.tile([128, 512], F32, tag="mm", name="mm")
                    for dk in range(4):
                        nc.tensor.matmul(
                            pt, WT[("Wk", dk)][:, nj * 128:(nj + 1) * 128],
                            s["kTg"][(g, dk)], start=(dk == 0), stop=(dk == 3))
                    nc.vector.tensor_copy(
                        s["kpTr"][nj][:, g * 512:(g + 1) * 512], pt)

                def c_vp(g, lb):
                    pt = mm_ps.tile([128, 512], F32, tag="mm", name="mm")
                    for dk in range(4):
                        nc.tensor.matmul(
                            pt, s["kTg"][(g, dk)][:, lb * 128:(lb + 1) * 128],
                            WT[("Wv", dk)], start=(dk == 0), stop=(dk == 3))
                    vt = vpap.tile([128, 520], BF16, tag="vpa", name="vpa")
                    vt3 = vt.rearrange("p (h c) -> p h c", c=65)
                    nc.vector.tensor_copy(
                        vt3[:, :, 0:64],
                        pt.rearrange("p (h c) -> p h c", c=64))
                    nc.vector.memset(vt3[:, :, 64:65], 1.0)
                    s["vpa"][g * 4 + lb] = vt

                def c_k2(g, nj):
                    if s["k2Tr"][nj] is None:
                        s["k2Tr"][nj] = pb.tile([128, LK], BF16, tag=f"k2t{nj}",
                                                name=f"k2t{nj}")
                    pt = mm_ps.tile([128, 512], F32, tag="mm", name="mm")
                    for dk in range(4):
                        nc.tensor.matmul(
                            pt, WT[("Wks", dk)][:, nj * 128:(nj + 1) * 128],
                            s["kTg"][(g, dk)], start=(dk == 0), stop=(dk == 3))
                    nc.scalar.activation(
                        s["k2Tr"][nj][:, g * 512:(g + 1) * 512], pt, AF.Copy)

                # k DMAs for the first two groups go to the very front so
                # the gpsimd queue starts them a full round early
                cl.insert(0, lambda: c_k_dma(0))
                cl.insert(2, lambda: c_k_dma(1))
                for g in range(NG):
                    if g + 2 < NG:
                        cl.append(lambda g=g: c_k_dma(g + 2))
                    for dk in range(4):
                        cl.append(lambda g=g, dk=dk: c_ktr(g, dk))
                    for nj in range(4):
                        cl.append(lambda g=g, nj=nj: c_kp(g, nj))
                    for lb in range(4):
                        cl.append(lambda g=g, lb=lb: c_vp(g, lb))
                    for nj in range(4):
                        cl.append(lambda g=g, nj=nj: c_k2(g, nj))
                return cl

            def f_out(pos):
                """out_proj, biasq, final scores for batch at pos."""
                bi = bis[pos]
                s = S(pos)
                cl = []

                def c_op(nj):
                    if "ncTr" not in s:
                        s["ncTr"] = [None] * 4
                    pt = mm_ps.tile([128, 512], F32, tag="mm", name="mm")
                    for t_ in range(4):
                        for hh in range(2):
                            nc.tensor.matmul(
                                pt[:, 0:256],
                                WoTh[2 * t_ + hh][:, nj * 128:(nj + 1) * 128],
                                s["ctx"][t_][:, hh * 256:(hh + 1) * 256],
                                start=(t_ == 0 and hh == 0),
                                stop=(t_ == 3 and hh == 1))
                    t = pb.tile([128, 256], BF16, tag=f"nct{nj}", name=f"nct{nj}")
                    nc.vector.tensor_scalar_add(out=t, in0=pt[:, 0:256],
                                                scalar1=bocol[nj][:, :])
                    s["ncTr"][nj] = t
                for nj in range(4):
                    cl.append(lambda nj=nj: c_op(nj))

                def c_biasq():
                    pt = mm_ps.tile([128, 512], F32, tag="mm", name="mm")
                    for nk in range(4):
                        nc.tensor.matmul(pt[0:1, 0:256], bksq[:, nk:nk + 1],
                                         s["ncTr"][nk],
                                         start=(nk == 0), stop=(nk == 3))
                    bqrow = smal.tile([1, 256], BF16, tag="bqrow", name="bqrow")
                    nc.vector.tensor_copy(bqrow, pt[0:1, 0:256])
                    ptt = tr_ps.tile([128, 512], BF16, tag="tr", name="tr")
                    for mi in range(2):
                        nc.tensor.transpose(ptt[:, 2 * mi:2 * mi + 1],
                                            bqrow[0:1, mi * 128:(mi + 1) * 128],
                                            ident[0:1, 0:1])
                    s["biasqT"] = pb.tile([128, 2], F32, tag="biasqT",
                                          name="biasqT")
                    for mi in range(2):
                        nc.vector.tensor_copy(s["biasqT"][:, mi:mi + 1],
                                              ptt[:, 2 * mi:2 * mi + 1])
                cl.append(c_biasq)

                def c_fin(mi, lg):
                    pt = mm_ps.tile([128, 512], F32, tag="mm", name="mm")
                    for nk in range(4):
                        nc.tensor.matmul(
                            pt, s["ncTr"][nk][:, mi * 128:(mi + 1) * 128],
                            s["k2Tr"][nk][:, lg * 512:(lg + 1) * 512],
                            start=(nk == 0), stop=(nk == 3))
                    th = thp.tile([128, 512], F32, tag="th", name="th")
                    nc.scalar.activation(th, pt, AF.Tanh,
                                         bias=s["biasqT"][:, mi:mi + 1],
                                         scale=ISQ_D)
                    th2 = thp.tile([128, 512], F32, tag="th2", name="th2")
                    nc.vector.tensor_scalar_mul(th2, th, CLIP)
                    nc.vector.copy_predicated(
                        th2, s["masku8"][:, lg * 512:(lg + 1) * 512],
                        fmin.to_broadcast([128, 512]))
                    nc.sync.dma_start(
                        out=out_d[bi, mi * 128:(mi + 1) * 128,
                                  lg * 512:(lg + 1) * 512],
                        in_=th2)
                for mi in range(2):
                    for lg in range(4):
                        cl.append(lambda mi=mi, lg=lg: c_fin(mi, lg))
                return cl

            # -------- attention unit machinery --------
            def emit_S(pos, t_, lc):
                s = S(pos)
                sp = st_ps.tile([128, 512], F32, tag="st", name="st")
                nc.tensor.matmul(
                    sp, s["kpTr"][t_][:, lc * 128:(lc + 1) * 128],
                    s["qpPad"][t_], start=True, stop=True)
                s[("sp", t_, lc)] = sp

            def emit_exp(pos, t_, lc):
                s = S(pos)
                et = etp.tile([128, 512], BF16, tag="et", name="et")
                nc.scalar.activation(
                    et, s.pop(("sp", t_, lc)), AF.Exp,
                    bias=s["mb"][:, lc:lc + 1], scale=ISQ_HD)
                s[("et", t_, lc)] = et

            def emit_U(pos, t_, lc):
                s = S(pos)
                if lc == 0:
                    s[("u", t_)] = u_ps.tile([128, 512], F32, tag="u", name="u")
                u = s[("u", t_)]
                et = s.pop(("et", t_, lc))
                vt = s["vpa"][lc]
                nc.tensor.matmul(
                    u[0:65, 0:256], vt[:, (2 * t_) * 65:(2 * t_) * 65 + 65],
                    et[:, 0:256], start=(lc == 0), stop=(lc == NLB - 1),
                    skip_group_check=True)
                # start=True on the even head cleared the whole bank; odd
                # head's first matmul relies on has_written=0 -> overwrite.
                nc.tensor.matmul(
                    u[0:65, 256:512],
                    vt[:, (2 * t_ + 1) * 65:(2 * t_ + 1) * 65 + 65],
                    et[:, 256:512], start=False, stop=(lc == NLB - 1),
                    skip_group_check=True)

            def emit_ctx(pos, t_):
                s = S(pos)
                u = s.pop(("u", t_))
                uf = smal.tile([128, 512], F32, tag="uf", name="uf")
                nc.vector.tensor_copy(uf[0:65, :], u[0:65, :])
                zr = smal.tile([1, 512], BF16, tag="zr", name="zr")
                nc.vector.tensor_copy(zr, uf[64:65, :])
                zb = mm_ps.tile([128, 512], F32, tag="mm", name="mm")
                nc.tensor.matmul(zb[0:64, :], onr[0:1, 0:64], zr,
                                 start=True, stop=True)
                rzt = smal.tile([64, 512], F32, tag="rzt", name="rzt")
                nc.vector.reciprocal_approx_fast(rzt, zb[0:64, :])
                if "ctx" not in s:
                    s["ctx"] = [None] * 4
                ct = pb.tile([64, 512], BF16, tag=f"ctx{t_}", name=f"ctx{t_}")
                nc.vector.tensor_mul(ct, uf[0:64, :], rzt)
                s["ctx"][t_] = ct

            def run_units(pos, fillers):
                # 2-deep S lookahead (3 st banks) + 1-deep Exp lookahead:
                # U(i) reads et(i) whose Exp was issued a full unit earlier.
                fill = deque(fillers)
                emit_S(pos, 0, 0)
                emit_exp(pos, 0, 0)
                for i in range(64):
                    t_, lc = divmod(i, 16)
                    if i + 1 < 64:
                        nt, nl = divmod(i + 1, 16)
                        emit_S(pos, nt, nl)
                        emit_exp(pos, nt, nl)
                    # fillers BEFORE U: PE chews projection work while ACT
                    # finishes Exp
                    n = -(-len(fill) // (64 - i))  # ceil
                    for _ in range(min(n, 4)):
                        if fill:
                            fill.popleft()()
                    emit_U(pos, t_, lc)
                    if lc == NLB - 1:
                        emit_ctx(pos, t_)
                while fill:
                    fill.popleft()()

            # ---------------- main schedule ----------------
            for clo in f_pre(0):
                clo()
            for pos in range(NB):
                fillers = []
                if pos > 0:
                    fillers += f_out(pos - 1)
                if pos + 1 < NB:
                    fillers += f_pre(pos + 1)
                run_units(pos, fillers)
                # free dead per-batch state
                if pos > 0:
                    st_.pop(pos - 1, None)
            for clo in f_out(NB - 1):
                clo()
    lowp.__exit__(None, None, None)
    nc.finalize()
    return nc


def kernel(**inputs):
    global LAST_RESULTS
    import os
    reps = int(os.environ.get("KERNEL_REPS", "1"))
    key = ("nc", reps)
    if key not in _CACHE:
        _CACHE[key] = _build(reps)
    nc = _CACHE[key]

    q = np.ascontiguousarray(np.asarray(inputs["q"], dtype=np.float32))
    k = np.ascontiguousarray(np.asarray(inputs["k"], dtype=np.float32))
    mask = np.ascontiguousarray(np.asarray(inputs["mask"]).astype(np.uint8))
    ws = {n: np.ascontiguousarray(np.asarray(inputs[n], dtype=np.float32))
          for n in W_NAMES}
    bs = {n: np.ascontiguousarray(np.asarray(inputs[n], dtype=np.float32))
          for n in ("bq", "bv", "bo", "bks")}

    in_maps = []
    for ci in range(NCORES):
        sl = slice(ci * BPC, (ci + 1) * BPC)
        im = {"q": q[sl], "k": k[sl], "mask": mask[sl]}
        im.update(ws)
        im.update(bs)
        in_maps.append(im)

    res = bass_utils.run_bass_kernel_spmd(
        nc, in_maps, core_ids=list(range(NCORES)), trace=TRACE)
    LAST_RESULTS = res
    out = np.concatenate([res.results[ci]["out"] for ci in range(NCORES)], axis=0)
    return out


# revision 26
# speedup vs baseline: 1.0559x; 1.0078x over previous
"""Trainium2 Bass kernel for nn_DecoderAttention (B=32, LQ=256, LK=2048, D=512, H=8).

Data-parallel over batch across 8 NeuronCores (4 batch items each).
All matmuls bf16. v2: software-pipelined emission keeps the PE saturated
and HAM-warm end to end.

Structure per batch b (steady state):
  64 "attention units" (t_ in 0..3 x lc in 0..15): S^T matmul (one
  [128,512] MM per lc via block-diag qpPad), ACT Exp (mask bias per
  l-partition), two U accumulation MMs ([v_e|1] rows 0:65 / [1|v_o] rows
  63:128 so ctx lands lane-aligned).  Between units, "filler" closures
  are consumed: out_proj/final-scores of batch b-1, k2 projection of b,
  and mask/q/qp/k-transpose/kp/vp of batch b+1.  The ACT-bound Exp
  stream thus always overlaps PE-bound projection work.

Algebraic folds (all exact):
  - bk dropped: adding a per-query constant to scores cancels in softmax.
  - bv folded into bo' = bo + Wo@bv.
  - bks folded into a per-q tanh bias: bias_q = ncT^T @ (bks/sqrt(D)).
PSUM: tr(2,bf16) + mm(2) + st(2) + u(2) = 8 banks.  mm rotation is
shared by qp/kp/vp/k2/out_proj/zb/biasq/final-score groups; st by S only.
Z reciprocal via reciprocal_approx_fast (5x faster than DVE reciprocal).
"""
import sys

sys.path.insert(0, "/opt/trn_rl_repo")

from collections import deque

import numpy as np

import concourse.bass as bass
import concourse.bacc as bacc
import concourse.mybir as mybir
import concourse.tile as tile
from concourse import bass_utils
from concourse.masks import make_identity

F32 = mybir.dt.float32
BF16 = mybir.dt.bfloat16
U8 = mybir.dt.uint8
AF = mybir.ActivationFunctionType

B, LQ, LK, D, H = 32, 256, 2048, 512, 8
HD = D // H              # 64
NCORES = 8
BPC = B // NCORES        # 4 batch items per core
NLB = LK // 128          # 16 l-blocks
NG = LK // 512           # 4 l-groups
CLIP = 10.0
FLOAT_MIN = -3.4e38
ISQ_HD = 0.125           # 1/sqrt(64)
ISQ_D = float(1.0 / np.sqrt(512.0))
MASK_BIG = -1e30
W_NAMES = ("Wq", "Wk", "Wv", "Wks", "Wo")
B_OF_W = {"Wq": "bq", "Wk": "bk", "Wv": "bv", "Wo": "bo", "Wks": "bks"}

TRACE = False
LAST_RESULTS = None
_CACHE = {}


def _build(reps=1):
    nc = bacc.Bacc("TRN2", target_bir_lowering=False, debug=False)
    q_d = nc.dram_tensor("q", [BPC, LQ, D], F32, kind="ExternalInput").ap()
    k_d = nc.dram_tensor("k", [BPC, LK, D], F32, kind="ExternalInput").ap()
    m_d = nc.dram_tensor("mask", [BPC, LK], U8, kind="ExternalInput").ap()
    w_d = {n: nc.dram_tensor(n, [D, D], F32, kind="ExternalInput").ap()
           for n in W_NAMES}
    # bk is unused (exact softmax cancellation) - not even declared.
    b_d = {n: nc.dram_tensor(n, [D], F32, kind="ExternalInput").ap()
           for n in ("bq", "bv", "bo", "bks")}
    out_d = nc.dram_tensor("out", [BPC, LQ, LK], F32, kind="ExternalOutput").ap()

    bis = [b for _ in range(reps) for b in range(BPC)]
    NB = len(bis)

    lowp = nc.allow_low_precision("bf16 matmul operands by design")
    lowp.__enter__()
    with tile.TileContext(nc) as tc:
        with (
            tc.tile_pool(name="c1", bufs=1) as c1,          # persistent consts
            tc.tile_pool(name="pb", bufs=2) as pb,          # per-batch persistents
            tc.tile_pool(name="vpap", bufs=32) as vpap,     # vp [v_e|1|1|v_o] tiles
            tc.tile_pool(name="knp", bufs=12) as knp,       # k/q/w natural staging
            tc.tile_pool(name="ktp", bufs=8) as ktp,        # kT group tiles
            tc.tile_pool(name="etp", bufs=4) as etp,        # exp output tiles
            tc.tile_pool(name="thp", bufs=3) as thp,        # final output staging
            tc.tile_pool(name="smal", bufs=2) as smal,      # small working tiles
            tc.tile_pool(name="tr_ps", bufs=2, space="PSUM") as tr_ps,
            tc.tile_pool(name="mm_ps", bufs=3, space="PSUM") as mm_ps,
            tc.tile_pool(name="st_ps", bufs=2, space="PSUM") as st_ps,
            tc.tile_pool(name="u_ps", bufs=1, space="PSUM") as u_ps,
        ):
            # ---------------- one-time setup ----------------
            # HAM warm-up first thing, while everything else boots
            wscr = c1.tile([128, 512], BF16, tag="wscr", name="wscr")
            nc.vector.memset(wscr, 1.0)
            for _wi in range(10):
                wp = mm_ps.tile([128, 512], F32, tag="mm", name="mm")
                nc.tensor.matmul(wp, wscr[:, 0:128], wscr, start=True, stop=True)
            identf = c1.tile([128, 128], F32, tag="identf", name="identf")
            make_identity(nc, identf)
            ident = c1.tile([128, 128], BF16, tag="ident", name="ident")
            nc.vector.tensor_copy(ident, identf)
            onr = c1.tile([1, 64], BF16, tag="onr", name="onr")
            nc.vector.memset(onr, 1.0)
            fmin = c1.tile([128, 1], F32, tag="fmin", name="fmin")
            nc.vector.memset(fmin, FLOAT_MIN)

            # bias columns
            bcol = {}
            for bn in ("bq", "bo", "bks", "bv"):
                ap3 = b_d[bn].rearrange("(c p one) -> c p one", p=128, one=1)
                for c in range(4):
                    t = c1.tile([128, 1], F32, tag=f"b_{bn}_{c}", name=f"b_{bn}_{c}")
                    nc.sync.dma_start(out=t, in_=ap3[c])
                    bcol[(bn, c)] = t
            # bks scaled by 1/sqrt(D), bf16 (for the biasq matmul)
            bksq = c1.tile([128, 4], BF16, tag="bksq", name="bksq")
            for c in range(4):
                nc.vector.tensor_scalar_mul(bksq[:, c:c + 1],
                                            bcol[("bks", c)], ISQ_D)
            # bv bf16 cols (for Wo@bv matmul)
            bvb = c1.tile([128, 4], BF16, tag="bvb", name="bvb")
            for c in range(4):
                nc.vector.tensor_copy(bvb[:, c:c + 1], bcol[("bv", c)])

            # transposed weights WT[(wn, dk)] = [128(din chunk), 512(dout)] bf16
            WT = {}
            drain_flip = [0]

            def drain_copy(dst, src):
                # alternate PSUM->SBUF drains between DVE and ACT
                if drain_flip[0] % 2 == 0:
                    nc.vector.tensor_copy(dst, src)
                else:
                    nc.scalar.activation(dst, src, AF.Copy)
                drain_flip[0] += 1

            for wn in ("Wk", "Wv", "Wq", "Wks", "Wo"):
                wnat = []
                for nj in range(4):
                    t = knp.tile([128, 512], BF16, tag="knat", name="knat")
                    nc.gpsimd.dma_start(
                        out=t, in_=w_d[wn][nj * 128:(nj + 1) * 128, :])
                    wnat.append(t)
                for dk in range(4):
                    pt = tr_ps.tile([128, 512], BF16, tag="tr", name="tr")
                    for nj in range(4):
                        nc.tensor.transpose(
                            pt[:, nj * 128:(nj + 1) * 128],
                            wnat[nj][:, dk * 128:(dk + 1) * 128], ident)
                    wt = c1.tile([128, 512], BF16, tag=f"wt_{wn}_{dk}",
                                 name=f"wt_{wn}_{dk}")
                    drain_copy(wt, pt)
                    WT[(wn, dk)] = wt
                if wn == "Wo":
                    # per-head WoTh[h] = [64(din in head), 512(dout)], base 0
                    WoTh = []
                    for h in range(H):
                        pt = tr_ps.tile([128, 512], BF16, tag="tr", name="tr")
                        for nj in range(4):
                            nc.tensor.transpose(
                                pt[0:64, nj * 128:(nj + 1) * 128],
                                wnat[nj][:, h * 64:(h + 1) * 64], ident)
                        wt = c1.tile([64, 512], BF16, tag=f"woth{h}",
                                     name=f"woth{h}")
                        drain_copy(wt, pt[0:64, :])
                        WoTh.append(wt)

            # bo' = bo + Wo@bv  (per-partition f32 cols)
            pt = mm_ps.tile([128, 512], F32, tag="mm", name="mm")
            for dk in range(4):
                nc.tensor.matmul(pt[0:1, :], bvb[:, dk:dk + 1], WT[("Wo", dk)],
                                 start=(dk == 0), stop=(dk == 3))
            borow = smal.tile([1, 512], BF16, tag="borow", name="borow")
            nc.vector.tensor_copy(borow, pt[0:1, :])
            ptt = tr_ps.tile([128, 512], BF16, tag="tr", name="tr")
            for nj in range(4):
                nc.tensor.transpose(ptt[:, 2 * nj:2 * nj + 1],
                                    borow[0:1, nj * 128:(nj + 1) * 128],
                                    ident[0:1, 0:1])
            wobvT = smal.tile([128, 4], F32, tag="wobvT", name="wobvT")
            for nj in range(4):
                nc.vector.tensor_copy(wobvT[:, nj:nj + 1],
                                      ptt[:, 2 * nj:2 * nj + 1])
            bocol = []
            for nj in range(4):
                t = c1.tile([128, 1], F32, tag=f"bop{nj}", name=f"bop{nj}")
                nc.vector.tensor_scalar_add(out=t, in0=wobvT[:, nj:nj + 1],
                                            scalar1=bcol[("bo", nj)][:, :])
                bocol.append(t)

            # ---------------- per-batch state ----------------
            st_ = {}  # pos -> dict of tiles

            def S(pos):
                if pos not in st_:
                    st_[pos] = {}
                return st_[pos]

            # -------- filler emitters (each returns list of closures) -----
            def f_pre(pos):
                """mask, q, qp, k DMA/transpose, kp, vp for batch at pos."""
                bi = bis[pos]
                s = S(pos)
                cl = []

                def c_mask_dma():
                    m_row = m_d[bi]
                    bcast = bass.AP(tensor=m_row.tensor, offset=m_row.offset,
                                    ap=[[0, 128]] + m_row.ap)
                    s["masku8"] = pb.tile([128, LK], U8, tag="masku8",
                                          name="masku8")
                    nc.gpsimd.dma_start(out=s["masku8"], in_=bcast)
                    s["m16"] = smal.tile([16, 128], U8, tag="m16", name="m16")
                    nc.sync.dma_start(
                        out=s["m16"], in_=m_row.rearrange("(c p) -> c p", c=16))
                cl.append(c_mask_dma)

                def c_mask_tr():
                    m16f = smal.tile([16, 128], BF16, tag="m16f", name="m16f")
                    nc.vector.tensor_copy(m16f, s["m16"])
                    mpt = tr_ps.tile([128, 512], BF16, tag="tr", name="tr")
                    nc.tensor.transpose(mpt[:, 0:16], m16f, ident[0:16, 0:16])
                    s["mb"] = pb.tile([128, 16], F32, tag="mb", name="mb")
                    nc.vector.tensor_scalar_mul(s["mb"], mpt[:, 0:16], MASK_BIG)
                cl.append(c_mask_tr)

                def c_q_dma():
                    s["qnat"] = []
                    for mi in range(2):
                        t = knp.tile([128, 512], BF16, tag="knat", name="knat")
                        nc.gpsimd.dma_start(
                            out=t, in_=q_d[bi, mi * 128:(mi + 1) * 128, :])
                        s["qnat"].append(t)
                cl.append(c_q_dma)

                def c_q_tr(dk):
                    if "qTr" not in s:
                        s["qTr"] = [None] * 4
                    pt = tr_ps.tile([128, 512], BF16, tag="tr", name="tr")
                    for mi in range(2):
                        nc.tensor.transpose(
                            pt[:, mi * 128:(mi + 1) * 128],
                            s["qnat"][mi][:, dk * 128:(dk + 1) * 128], ident)
                    t = pb.tile([128, 256], BF16, tag=f"qtr{dk}", name=f"qtr{dk}")
                    nc.vector.tensor_copy(t, pt[:, 0:256])
                    s["qTr"][dk] = t
                for dk in range(4):
                    cl.append(lambda dk=dk: c_q_tr(dk))

                def c_qp(nj):
                    if "qpPad" not in s:
                        s["qpPad"] = [None] * 4
                    pt = mm_ps.tile([128, 512], F32, tag="mm", name="mm")
                    for dk in range(4):
                        nc.tensor.matmul(
                            pt[:, 0:256],
                            WT[("Wq", dk)][:, nj * 128:(nj + 1) * 128],
                            s["qTr"][dk], start=(dk == 0), stop=(dk == 3))
                    t = pb.tile([128, 512], BF16, tag=f"qpd{nj}", name=f"qpd{nj}")
                    nc.vector.memset(t, 0.0)
                    nc.vector.tensor_scalar_add(
                        out=t[0:64, 0:256], in0=pt[0:64, 0:256],
                        scalar1=bcol[("bq", nj)][0:64, :])
                    nc.vector.tensor_scalar_add(
                        out=t[64:128, 256:512], in0=pt[64:128, 0:256],
                        scalar1=bcol[("bq", nj)][64:128, :])
                    s["qpPad"][nj] = t
                for nj in range(4):
                    cl.append(lambda nj=nj: c_qp(nj))

                s["kpTr"] = [None] * 4
                s["k2Tr"] = [None] * 4
                s["vpa"] = [None] * NLB
                s["kTg"] = {}
                s["knat"] = {}

                def c_k_dma(g):
                    knat = []
                    lbase = g * 512
                    for li in range(4):
                        t = knp.tile([128, 512], BF16, tag="knat", name="knat")
                        nc.gpsimd.dma_start(
                            out=t,
                            in_=k_d[bi, lbase + li * 128:lbase + (li + 1) * 128, :])
                        knat.append(t)
                    s["knat"][g] = knat

                def c_ktr(g, dk):
                    pt = tr_ps.tile([128, 512], BF16, tag="tr", name="tr")
                    for li in range(4):
                        nc.tensor.transpose(
                            pt[:, li * 128:(li + 1) * 128],
                            s["knat"][g][li][:, dk * 128:(dk + 1) * 128], ident)
                    t = ktp.tile([128, 512], BF16, tag="ktg", name="ktg")
                    nc.vector.tensor_copy(t, pt)
                    s["kTg"][(g, dk)] = t

                def c_kp(g, nj):
                    if s["kpTr"][nj] is None:
                        s["kpTr"][nj] = pb.tile([128, LK], BF16, tag=f"kpt{nj}",
                                                name=f"kpt{nj}")
                    pt = mm_ps.tile([128, 512], F32, tag="mm", name="mm")
                    for dk in range(4):
                        nc.tensor.matmul(
                            pt, WT[("Wk", dk)][:, nj * 128:(nj + 1) * 128],
                            s["kTg"][(g, dk)], start=(dk == 0), stop=(dk == 3))
                    nc.vector.tensor_copy(
                        s["kpTr"][nj][:, g * 512:(g + 1) * 512], pt)

                def c_vp(g, lb):
                    pt = mm_ps.tile([128, 512], F32, tag="mm", name="mm")
                    for dk in range(4):
                        nc.tensor.matmul(
                            pt, s["kTg"][(g, dk)][:, lb * 128:(lb + 1) * 128],
                            WT[("Wv", dk)], start=(dk == 0), stop=(dk == 3))
                    vt = vpap.tile([128, 520], BF16, tag="vpa", name="vpa")
                    vt3 = vt.rearrange("p (h c) -> p h c", c=65)
                    nc.vector.tensor_copy(
                        vt3[:, :, 0:64],
                        pt.rearrange("p (h c) -> p h c", c=64))
                    nc.vector.memset(vt3[:, :, 64:65], 1.0)
                    s["vpa"][g * 4 + lb] = vt

                def c_k2(g, nj):
                    if s["k2Tr"][nj] is None:
                        s["k2Tr"][nj] = pb.tile([128, LK], BF16, tag=f"k2t{nj}",
                                                name=f"k2t{nj}")
                    pt = mm_ps.tile([128, 512], F32, tag="mm", name="mm")
                    for dk in range(4):
                        nc.tensor.matmul(
                            pt, WT[("Wks", dk)][:, nj * 128:(nj + 1) * 128],
                            s["kTg"][(g, dk)], start=(dk == 0), stop=(dk == 3))
                    nc.scalar.activation(
                        s["k2Tr"][nj][:, g * 512:(g + 1) * 512], pt, AF.Copy)

                # k DMAs for the first two groups go to the very front so
                # the gpsimd queue starts them a full round early
                cl.insert(0, lambda: c_k_dma(0))
                cl.insert(2, lambda: c_k_dma(1))
                for g in range(NG):
                    if g + 2 < NG:
                        cl.append(lambda g=g: c_k_dma(g + 2))
                    for dk in range(4):
                        cl.append(lambda g=g, dk=dk: c_ktr(g, dk))
                    for nj in range(4):
                        cl.append(lambda g=g, nj=nj: c_kp(g, nj))
                    for lb in range(4):
                        cl.append(lambda g=g, lb=lb: c_vp(g, lb))
                    for nj in range(4):
                        cl.append(lambda g=g, nj=nj: c_k2(g, nj))
                return cl

            def f_out(pos):
                """out_proj, biasq, final scores for batch at pos."""
                bi = bis[pos]
                s = S(pos)
                cl = []

                def c_op(nj):
                    if "ncTr" not in s:
                        s["ncTr"] = [None] * 4
                    pt = mm_ps.tile([128, 512], F32, tag="mm", name="mm")
                    for t_ in range(4):
                        for hh in range(2):
                            nc.tensor.matmul(
                                pt[:, 0:256],
                                WoTh[2 * t_ + hh][:, nj * 128:(nj + 1) * 128],
                                s["ctx"][t_][:, hh * 256:(hh + 1) * 256],
                                start=(t_ == 0 and hh == 0),
                                stop=(t_ == 3 and hh == 1))
                    t = pb.tile([128, 256], BF16, tag=f"nct{nj}", name=f"nct{nj}")
                    nc.vector.tensor_scalar_add(out=t, in0=pt[:, 0:256],
                                                scalar1=bocol[nj][:, :])
                    s["ncTr"][nj] = t
                for nj in range(4):
                    cl.append(lambda nj=nj: c_op(nj))

                def c_biasq():
                    pt = mm_ps.tile([128, 512], F32, tag="mm", name="mm")
                    for nk in range(4):
                        nc.tensor.matmul(pt[0:1, 0:256], bksq[:, nk:nk + 1],
                                         s["ncTr"][nk],
                                         start=(nk == 0), stop=(nk == 3))
                    bqrow = smal.tile([1, 256], BF16, tag="bqrow", name="bqrow")
                    nc.vector.tensor_copy(bqrow, pt[0:1, 0:256])
                    ptt = tr_ps.tile([128, 512], BF16, tag="tr", name="tr")
                    for mi in range(2):
                        nc.tensor.transpose(ptt[:, 2 * mi:2 * mi + 1],
                                            bqrow[0:1, mi * 128:(mi + 1) * 128],
                                            ident[0:1, 0:1])
                    s["biasqT"] = pb.tile([128, 2], F32, tag="biasqT",
                                          name="biasqT")
                    for mi in range(2):
                        nc.vector.tensor_copy(s["biasqT"][:, mi:mi + 1],
                                              ptt[:, 2 * mi:2 * mi + 1])
                cl.append(c_biasq)

                def c_fin(mi, lg):
                    pt = mm_ps.tile([128, 512], F32, tag="mm", name="mm")
                    for nk in range(4):
                        nc.tensor.matmul(
                            pt, s["ncTr"][nk][:, mi * 128:(mi + 1) * 128],
                            s["k2Tr"][nk][:, lg * 512:(lg + 1) * 512],
                            start=(nk == 0), stop=(nk == 3))
                    th = thp.tile([128, 512], F32, tag="th", name="th")
                    nc.scalar.activation(th, pt, AF.Tanh,
                                         bias=s["biasqT"][:, mi:mi + 1],
                                         scale=ISQ_D)
                    th2 = thp.tile([128, 512], F32, tag="th2", name="th2")
                    nc.vector.tensor_scalar_mul(th2, th, CLIP)
                    nc.vector.copy_predicated(
                        th2, s["masku8"][:, lg * 512:(lg + 1) * 512],
                        fmin.to_broadcast([128, 512]))
                    nc.sync.dma_start(
                        out=out_d[bi, mi * 128:(mi + 1) * 128,
                                  lg * 512:(lg + 1) * 512],
                        in_=th2)
                for mi in range(2):
                    for lg in range(4):
                        cl.append(lambda mi=mi, lg=lg: c_fin(mi, lg))
                return cl

            # -------- attention unit machinery --------
            def emit_S(pos, t_, lc):
                s = S(pos)
                sp = st_ps.tile([128, 512], F32, tag="st", name="st")
                nc.tensor.matmul(
                    sp, s["kpTr"][t_][:, lc * 128:(lc + 1) * 128],
                    s["qpPad"][t_], start=True, stop=True)
                s[("sp", t_, lc)] = sp

            def emit_exp(pos, t_, lc):
                s = S(pos)
                et = etp.tile([128, 512], BF16, tag="et", name="et")
                nc.scalar.activation(
                    et, s.pop(("sp", t_, lc)), AF.Exp,
                    bias=s["mb"][:, lc:lc + 1], scale=ISQ_HD)
                s[("et", t_, lc)] = et

            def emit_U(pos, t_, lc):
                s = S(pos)
                if lc == 0:
                    s[("u", t_)] = u_ps.tile([128, 512], F32, tag="u", name="u")
                u = s[("u", t_)]
                et = s.pop(("et", t_, lc))
                vt = s["vpa"][lc]
                nc.tensor.matmul(
                    u[0:65, 0:256], vt[:, (2 * t_) * 65:(2 * t_) * 65 + 65],
                    et[:, 0:256], start=(lc == 0), stop=(lc == NLB - 1),
                    skip_group_check=True)
                # start=True on the even head cleared the whole bank; odd
                # head's first matmul relies on has_written=0 -> overwrite.
                nc.tensor.matmul(
                    u[0:65, 256:512],
                    vt[:, (2 * t_ + 1) * 65:(2 * t_ + 1) * 65 + 65],
                    et[:, 256:512], start=False, stop=(lc == NLB - 1),
                    skip_group_check=True)

            def emit_ctx(pos, t_):
                s = S(pos)
                u = s.pop(("u", t_))
                uf = smal.tile([128, 512], F32, tag="uf", name="uf")
                nc.vector.tensor_copy(uf[0:65, :], u[0:65, :])
                zr = smal.tile([1, 512], BF16, tag="zr", name="zr")
                nc.vector.tensor_copy(zr, uf[64:65, :])
                zb = mm_ps.tile([128, 512], F32, tag="mm", name="mm")
                nc.tensor.matmul(zb[0:64, :], onr[0:1, 0:64], zr,
                                 start=True, stop=True)
                rzt = smal.tile([64, 512], F32, tag="rzt", name="rzt")
                nc.vector.reciprocal_approx_fast(rzt, zb[0:64, :])
                if "ctx" not in s:
                    s["ctx"] = [None] * 4
                ct = pb.tile([64, 512], BF16, tag=f"ctx{t_}", name=f"ctx{t_}")
                nc.vector.tensor_mul(ct, uf[0:64, :], rzt)
                s["ctx"][t_] = ct

            def run_units(pos, fillers):
                # 2-deep S lookahead (3 st banks) + 1-deep Exp lookahead:
                # U(i) reads et(i) whose Exp was issued a full unit earlier.
                fill = deque(fillers)
                emit_S(pos, 0, 0)
                emit_exp(pos, 0, 0)
                for i in range(64):
                    t_, lc = divmod(i, 16)
                    if i + 1 < 64:
                        nt, nl = divmod(i + 1, 16)
                        emit_S(pos, nt, nl)
                        emit_exp(pos, nt, nl)
                    # fillers BEFORE U: PE chews projection work while ACT
                    # finishes Exp
                    n = -(-len(fill) // (64 - i))  # ceil
                    for _ in range(min(n, 4)):
                        if fill:
                            fill.popleft()()
                    emit_U(pos, t_, lc)
                    if lc == NLB - 1:
                        emit_ctx(pos, t_)
                while fill:
                    fill.popleft()()

            # ---------------- main schedule ----------------
            for clo in f_pre(0):
                clo()
            for pos in range(NB):
                fillers = []
                if pos > 0:
                    fillers += f_out(pos - 1)
                if pos + 1 < NB:
                    fillers += f_pre(pos + 1)
                run_units(pos, fillers)
                # free dead per-batch state
                if pos > 0:
                    st_.pop(pos - 1, None)
            for clo in f_out(NB - 1):
                clo()
    lowp.__exit__(None, None, None)
    nc.finalize()
    return nc


def kernel(**inputs):
    global LAST_RESULTS
    import os
    reps = int(os.environ.get("KERNEL_REPS", "1"))
    key = ("nc", reps)
    if key not in _CACHE:
        _CACHE[key] = _build(reps)
    nc = _CACHE[key]

    q = np.ascontiguousarray(np.asarray(inputs["q"], dtype=np.float32))
    k = np.ascontiguousarray(np.asarray(inputs["k"], dtype=np.float32))
    mask = np.ascontiguousarray(np.asarray(inputs["mask"]).astype(np.uint8))
    ws = {n: np.ascontiguousarray(np.asarray(inputs[n], dtype=np.float32))
          for n in W_NAMES}
    bs = {n: np.ascontiguousarray(np.asarray(inputs[n], dtype=np.float32))
          for n in ("bq", "bv", "bo", "bks")}

    in_maps = []
    for ci in range(NCORES):
        sl = slice(ci * BPC, (ci + 1) * BPC)
        im = {"q": q[sl], "k": k[sl], "mask": mask[sl]}
        im.update(ws)
        im.update(bs)
        in_maps.append(im)

    res = bass_utils.run_bass_kernel_spmd(
        nc, in_maps, core_ids=list(range(NCORES)), trace=TRACE)
    LAST_RESULTS = res
    out = np.concatenate([res.results[ci]["out"] for ci in range(NCORES)], axis=0)
    return out


# revision 32
# speedup vs baseline: 1.1769x; 1.1146x over previous
"""Trainium2 Bass kernel for nn_DecoderAttention (B=32, LQ=256, LK=2048, D=512, H=8).

Data-parallel over batch across 8 NeuronCores (4 batch items each).
All matmuls bf16. v2: software-pipelined emission keeps the PE saturated
and HAM-warm end to end.

Structure per batch b (steady state):
  64 "attention units" (t_ in 0..3 x lc in 0..15): S^T matmul (one
  [128,512] MM per lc via block-diag qpPad), ACT Exp (mask bias per
  l-partition), two U accumulation MMs ([v_e|1] rows 0:65 / [1|v_o] rows
  63:128 so ctx lands lane-aligned).  Between units, "filler" closures
  are consumed: out_proj/final-scores of batch b-1, k2 projection of b,
  and mask/q/qp/k-transpose/kp/vp of batch b+1.  The ACT-bound Exp
  stream thus always overlaps PE-bound projection work.

Algebraic folds (all exact):
  - bk dropped: adding a per-query constant to scores cancels in softmax.
  - bv folded into bo' = bo + Wo@bv.
  - bks folded into a per-q tanh bias: bias_q = ncT^T @ (bks/sqrt(D)).
PSUM: tr(2,bf16) + mm(2) + st(2) + u(2) = 8 banks.  mm rotation is
shared by qp/kp/vp/k2/out_proj/zb/biasq/final-score groups; st by S only.
Z reciprocal via reciprocal_approx_fast (5x faster than DVE reciprocal).
"""
import sys

sys.path.insert(0, "/opt/trn_rl_repo")

from collections import deque

import numpy as np

import concourse.bass as bass
import concourse.bacc as bacc
import concourse.mybir as mybir
import concourse.tile as tile
from concourse import bass_utils
from concourse.masks import make_identity

F32 = mybir.dt.float32
BF16 = mybir.dt.bfloat16
U8 = mybir.dt.uint8
AF = mybir.ActivationFunctionType

B, LQ, LK, D, H = 32, 256, 2048, 512, 8
HD = D // H              # 64
NCORES = 8
BPC = B // NCORES        # 4 batch items per core
NLB = LK // 128          # 16 l-blocks
NG = LK // 512           # 4 l-groups
CLIP = 10.0
FLOAT_MIN = -3.4e38
ISQ_HD = 0.125           # 1/sqrt(64)
ISQ_D = float(1.0 / np.sqrt(512.0))
MASK_BIG = -1e30
W_NAMES = ("Wq", "Wk", "Wv", "Wks", "Wo")
B_OF_W = {"Wq": "bq", "Wk": "bk", "Wv": "bv", "Wo": "bo", "Wks": "bks"}

TRACE = False
LAST_RESULTS = None
_CACHE = {}


def _build(reps=1):
    nc = bacc.Bacc("TRN2", target_bir_lowering=False, debug=False)
    q_d = nc.dram_tensor("q", [BPC, LQ, D], F32, kind="ExternalInput").ap()
    k_d = nc.dram_tensor("k", [BPC, LK, D], F32, kind="ExternalInput").ap()
    m_d = nc.dram_tensor("mask", [BPC, LK], U8, kind="ExternalInput").ap()
    w_d = {n: nc.dram_tensor(n, [D, D], F32, kind="ExternalInput").ap()
           for n in W_NAMES}
    # bk is unused (exact softmax cancellation) - not even declared.
    b_d = {n: nc.dram_tensor(n, [D], F32, kind="ExternalInput").ap()
           for n in ("bq", "bv", "bo", "bks")}
    out_d = nc.dram_tensor("out", [BPC, LQ, LK], F32, kind="ExternalOutput").ap()

    bis = [b for _ in range(reps) for b in range(BPC)]
    NB = len(bis)

    lowp = nc.allow_low_precision("bf16 matmul operands by design")
    lowp.__enter__()
    with tile.TileContext(nc) as tc:
        with (
            tc.tile_pool(name="c1", bufs=1) as c1,          # persistent consts
            tc.tile_pool(name="pb", bufs=2) as pb,          # per-batch persistents
            tc.tile_pool(name="vpap", bufs=32) as vpap,     # vp [v|1] tiles
            tc.tile_pool(name="knp", bufs=12) as knp,       # k/q/w natural staging
            tc.tile_pool(name="etp", bufs=4) as etp,        # exp output tiles
            tc.tile_pool(name="thp", bufs=3) as thp,        # final output staging
            tc.tile_pool(name="smal", bufs=2) as smal,      # small working tiles
            tc.tile_pool(name="tr_ps", bufs=2, space="PSUM") as tr_ps,
            tc.tile_pool(name="mm_ps", bufs=3, space="PSUM") as mm_ps,
            tc.tile_pool(name="st_ps", bufs=2, space="PSUM") as st_ps,
            tc.tile_pool(name="u_ps", bufs=1, space="PSUM") as u_ps,
        ):
            # ---------------- one-time setup ----------------
            # HAM warm-up first thing, while everything else boots
            wscr = c1.tile([128, 512], BF16, tag="wscr", name="wscr")
            nc.vector.memset(wscr, 1.0)
            for _wi in range(10):
                wp = mm_ps.tile([128, 512], F32, tag="mm", name="mm")
                nc.tensor.matmul(wp, wscr[:, 0:128], wscr, start=True, stop=True)
            identf = c1.tile([128, 128], F32, tag="identf", name="identf")
            make_identity(nc, identf)
            ident = c1.tile([128, 128], BF16, tag="ident", name="ident")
            nc.vector.tensor_copy(ident, identf)
            onr = c1.tile([1, 64], BF16, tag="onr", name="onr")
            nc.vector.memset(onr, 1.0)
            fmin = c1.tile([128, 1], F32, tag="fmin", name="fmin")
            nc.vector.memset(fmin, FLOAT_MIN)

            # bias columns
            bcol = {}
            for bn in ("bq", "bo", "bks", "bv"):
                ap3 = b_d[bn].rearrange("(c p one) -> c p one", p=128, one=1)
                for c in range(4):
                    t = c1.tile([128, 1], F32, tag=f"b_{bn}_{c}", name=f"b_{bn}_{c}")
                    nc.sync.dma_start(out=t, in_=ap3[c])
                    bcol[(bn, c)] = t
            # bks scaled by 1/sqrt(D), bf16 (for the biasq matmul)
            bksq = c1.tile([128, 4], BF16, tag="bksq", name="bksq")
            for c in range(4):
                nc.vector.tensor_scalar_mul(bksq[:, c:c + 1],
                                            bcol[("bks", c)], ISQ_D)
            # bv bf16 cols (for Wo@bv matmul)
            bvb = c1.tile([128, 4], BF16, tag="bvb", name="bvb")
            for c in range(4):
                nc.vector.tensor_copy(bvb[:, c:c + 1], bcol[("bv", c)])

            # transposed weights WT[(wn, dk)] = [128(din chunk), 512(dout)] bf16
            WT = {}
            drain_flip = [0]

            def drain_copy(dst, src):
                # alternate PSUM->SBUF drains between DVE and ACT
                if drain_flip[0] % 2 == 0:
                    nc.vector.tensor_copy(dst, src)
                else:
                    nc.scalar.activation(dst, src, AF.Copy)
                drain_flip[0] += 1

            # Wks stays NATURAL: the final scores use the associativity
            # nc @ (k Wks^T)^T == (nc Wks) @ k^T, so only nc (256 rows)
            # is projected, never k (2048 rows).
            wksN = []
            for nj in range(4):
                t = c1.tile([128, 512], BF16, tag=f"wksn{nj}", name=f"wksn{nj}")
                nc.gpsimd.dma_start(
                    out=t, in_=w_d["Wks"][nj * 128:(nj + 1) * 128, :])
                wksN.append(t)
            for wn in ("Wk", "Wv", "Wq", "Wo"):
                wnat = []
                for nj in range(4):
                    t = knp.tile([128, 512], BF16, tag="knat", name="knat")
                    nc.gpsimd.dma_start(
                        out=t, in_=w_d[wn][nj * 128:(nj + 1) * 128, :])
                    wnat.append(t)
                for dk in range(4):
                    pt = tr_ps.tile([128, 512], BF16, tag="tr", name="tr")
                    for nj in range(4):
                        nc.tensor.transpose(
                            pt[:, nj * 128:(nj + 1) * 128],
                            wnat[nj][:, dk * 128:(dk + 1) * 128], ident)
                    wt = c1.tile([128, 512], BF16, tag=f"wt_{wn}_{dk}",
                                 name=f"wt_{wn}_{dk}")
                    drain_copy(wt, pt)
                    WT[(wn, dk)] = wt
                if wn == "Wo":
                    # per-head WoTh[h] = [64(din in head), 512(dout)], base 0
                    WoTh = []
                    for h in range(H):
                        pt = tr_ps.tile([128, 512], BF16, tag="tr", name="tr")
                        for nj in range(4):
                            nc.tensor.transpose(
                                pt[0:64, nj * 128:(nj + 1) * 128],
                                wnat[nj][:, h * 64:(h + 1) * 64], ident)
                        wt = c1.tile([64, 512], BF16, tag=f"woth{h}",
                                     name=f"woth{h}")
                        drain_copy(wt, pt[0:64, :])
                        WoTh.append(wt)

            # bo' = bo + Wo@bv  (per-partition f32 cols)
            pt = mm_ps.tile([128, 512], F32, tag="mm", name="mm")
            for dk in range(4):
                nc.tensor.matmul(pt[0:1, :], bvb[:, dk:dk + 1], WT[("Wo", dk)],
                                 start=(dk == 0), stop=(dk == 3))
            borow = smal.tile([1, 512], BF16, tag="borow", name="borow")
            nc.vector.tensor_copy(borow, pt[0:1, :])
            ptt = tr_ps.tile([128, 512], BF16, tag="tr", name="tr")
            for nj in range(4):
                nc.tensor.transpose(ptt[:, 2 * nj:2 * nj + 1],
                                    borow[0:1, nj * 128:(nj + 1) * 128],
                                    ident[0:1, 0:1])
            wobvT = smal.tile([128, 4], F32, tag="wobvT", name="wobvT")
            for nj in range(4):
                nc.vector.tensor_copy(wobvT[:, nj:nj + 1],
                                      ptt[:, 2 * nj:2 * nj + 1])
            bocol = []
            for nj in range(4):
                t = c1.tile([128, 1], F32, tag=f"bop{nj}", name=f"bop{nj}")
                nc.vector.tensor_scalar_add(out=t, in0=wobvT[:, nj:nj + 1],
                                            scalar1=bcol[("bo", nj)][:, :])
                bocol.append(t)

            # ---------------- per-batch state ----------------
            st_ = {}  # pos -> dict of tiles

            def S(pos):
                if pos not in st_:
                    st_[pos] = {}
                return st_[pos]

            # -------- filler emitters (each returns list of closures) -----
            def f_pre(pos):
                """mask, q, qp, k DMA/transpose, kp, vp for batch at pos."""
                bi = bis[pos]
                s = S(pos)
                cl = []

                def c_mask_dma():
                    m_row = m_d[bi]
                    bcast = bass.AP(tensor=m_row.tensor, offset=m_row.offset,
                                    ap=[[0, 128]] + m_row.ap)
                    s["masku8"] = pb.tile([128, LK], U8, tag="masku8",
                                          name="masku8")
                    nc.gpsimd.dma_start(out=s["masku8"], in_=bcast)
                    s["m16"] = smal.tile([16, 128], U8, tag="m16", name="m16")
                    nc.sync.dma_start(
                        out=s["m16"], in_=m_row.rearrange("(c p) -> c p", c=16))
                cl.append(c_mask_dma)

                def c_mask_tr():
                    m16f = smal.tile([16, 128], BF16, tag="m16f", name="m16f")
                    nc.vector.tensor_copy(m16f, s["m16"])
                    mpt = tr_ps.tile([128, 512], BF16, tag="tr", name="tr")
                    nc.tensor.transpose(mpt[:, 0:16], m16f, ident[0:16, 0:16])
                    s["mb"] = pb.tile([128, 16], F32, tag="mb", name="mb")
                    nc.vector.tensor_scalar_mul(s["mb"], mpt[:, 0:16], MASK_BIG)
                cl.append(c_mask_tr)

                def c_q_dma():
                    s["qnat"] = []
                    for mi in range(2):
                        t = knp.tile([128, 512], BF16, tag="knat", name="knat")
                        nc.gpsimd.dma_start(
                            out=t, in_=q_d[bi, mi * 128:(mi + 1) * 128, :])
                        s["qnat"].append(t)
                cl.append(c_q_dma)

                def c_q_tr(dk):
                    if "qTr" not in s:
                        s["qTr"] = [None] * 4
                    pt = tr_ps.tile([128, 512], BF16, tag="tr", name="tr")
                    for mi in range(2):
                        nc.tensor.transpose(
                            pt[:, mi * 128:(mi + 1) * 128],
                            s["qnat"][mi][:, dk * 128:(dk + 1) * 128], ident)
                    t = pb.tile([128, 256], BF16, tag=f"qtr{dk}", name=f"qtr{dk}")
                    nc.vector.tensor_copy(t, pt[:, 0:256])
                    s["qTr"][dk] = t
                for dk in range(4):
                    cl.append(lambda dk=dk: c_q_tr(dk))

                def c_qp(nj):
                    if "qpPad" not in s:
                        s["qpPad"] = [None] * 4
                    pt = mm_ps.tile([128, 512], F32, tag="mm", name="mm")
                    for dk in range(4):
                        nc.tensor.matmul(
                            pt[:, 0:256],
                            WT[("Wq", dk)][:, nj * 128:(nj + 1) * 128],
                            s["qTr"][dk], start=(dk == 0), stop=(dk == 3))
                    t = pb.tile([128, 512], BF16, tag=f"qpd{nj}", name=f"qpd{nj}")
                    nc.vector.memset(t, 0.0)
                    nc.vector.tensor_scalar_add(
                        out=t[0:64, 0:256], in0=pt[0:64, 0:256],
                        scalar1=bcol[("bq", nj)][0:64, :])
                    nc.vector.tensor_scalar_add(
                        out=t[64:128, 256:512], in0=pt[64:128, 0:256],
                        scalar1=bcol[("bq", nj)][64:128, :])
                    s["qpPad"][nj] = t
                for nj in range(4):
                    cl.append(lambda nj=nj: c_qp(nj))

                s["kpTr"] = [None] * 4
                s["kTT"] = [None] * 4
                s["vpa"] = [None] * NLB
                s["knat"] = {}

                def c_k_dma(g):
                    knat = []
                    lbase = g * 512
                    for li in range(4):
                        t = knp.tile([128, 512], BF16, tag="knat", name="knat")
                        nc.gpsimd.dma_start(
                            out=t,
                            in_=k_d[bi, lbase + li * 128:lbase + (li + 1) * 128, :])
                        knat.append(t)
                    s["knat"][g] = knat

                def c_ktr(g, dk):
                    if s["kTT"][dk] is None:
                        s["kTT"][dk] = pb.tile([128, LK], BF16, tag=f"kt{dk}",
                                               name=f"kt{dk}")
                    pt = tr_ps.tile([128, 512], BF16, tag="tr", name="tr")
                    for li in range(4):
                        nc.tensor.transpose(
                            pt[:, li * 128:(li + 1) * 128],
                            s["knat"][g][li][:, dk * 128:(dk + 1) * 128], ident)
                    nc.vector.tensor_copy(
                        s["kTT"][dk][:, g * 512:(g + 1) * 512], pt)

                def c_kp(g, nj):
                    if s["kpTr"][nj] is None:
                        s["kpTr"][nj] = pb.tile([128, LK], BF16, tag=f"kpt{nj}",
                                                name=f"kpt{nj}")
                    pt = mm_ps.tile([128, 512], F32, tag="mm", name="mm")
                    for dk in range(4):
                        nc.tensor.matmul(
                            pt, WT[("Wk", dk)][:, nj * 128:(nj + 1) * 128],
                            s["kTT"][dk][:, g * 512:(g + 1) * 512],
                            start=(dk == 0), stop=(dk == 3))
                    nc.vector.tensor_copy(
                        s["kpTr"][nj][:, g * 512:(g + 1) * 512], pt)

                def c_vp(g, lb):
                    pt = mm_ps.tile([128, 512], F32, tag="mm", name="mm")
                    for dk in range(4):
                        nc.tensor.matmul(
                            pt,
                            s["kTT"][dk][:, g * 512 + lb * 128:
                                         g * 512 + (lb + 1) * 128],
                            WT[("Wv", dk)], start=(dk == 0), stop=(dk == 3))
                    vt = vpap.tile([128, 520], BF16, tag="vpa", name="vpa")
                    vt3 = vt.rearrange("p (h c) -> p h c", c=65)
                    nc.vector.tensor_copy(
                        vt3[:, :, 0:64],
                        pt.rearrange("p (h c) -> p h c", c=64))
                    nc.vector.memset(vt3[:, :, 64:65], 1.0)
                    s["vpa"][g * 4 + lb] = vt

                # k DMAs for the first two groups go to the very front so
                # the gpsimd queue starts them a full round early
                cl.insert(0, lambda: c_k_dma(0))
                cl.insert(2, lambda: c_k_dma(1))
                for g in range(NG):
                    if g + 2 < NG:
                        cl.append(lambda g=g: c_k_dma(g + 2))
                    for dk in range(4):
                        cl.append(lambda g=g, dk=dk: c_ktr(g, dk))
                    for nj in range(4):
                        cl.append(lambda g=g, nj=nj: c_kp(g, nj))
                    for lb in range(4):
                        cl.append(lambda g=g, lb=lb: c_vp(g, lb))
                return cl

            def f_out(pos):
                """out_proj, biasq, final scores for batch at pos."""
                bi = bis[pos]
                s = S(pos)
                cl = []

                def c_op(nj):
                    if "ncTr" not in s:
                        s["ncTr"] = [None] * 4
                    pt = mm_ps.tile([128, 512], F32, tag="mm", name="mm")
                    for t_ in range(4):
                        for hh in range(2):
                            nc.tensor.matmul(
                                pt[:, 0:256],
                                WoTh[2 * t_ + hh][:, nj * 128:(nj + 1) * 128],
                                s["ctx"][t_][:, hh * 256:(hh + 1) * 256],
                                start=(t_ == 0 and hh == 0),
                                stop=(t_ == 3 and hh == 1))
                    t = pb.tile([128, 256], BF16, tag=f"nct{nj}", name=f"nct{nj}")
                    nc.vector.tensor_scalar_add(out=t, in0=pt[:, 0:256],
                                                scalar1=bocol[nj][:, :])
                    s["ncTr"][nj] = t
                for nj in range(4):
                    cl.append(lambda nj=nj: c_op(nj))

                def c_biasq():
                    pt = mm_ps.tile([128, 512], F32, tag="mm", name="mm")
                    for nk in range(4):
                        nc.tensor.matmul(pt[0:1, 0:256], bksq[:, nk:nk + 1],
                                         s["ncTr"][nk],
                                         start=(nk == 0), stop=(nk == 3))
                    bqrow = smal.tile([1, 256], BF16, tag="bqrow", name="bqrow")
                    nc.vector.tensor_copy(bqrow, pt[0:1, 0:256])
                    ptt = tr_ps.tile([128, 512], BF16, tag="tr", name="tr")
                    for mi in range(2):
                        nc.tensor.transpose(ptt[:, 2 * mi:2 * mi + 1],
                                            bqrow[0:1, mi * 128:(mi + 1) * 128],
                                            ident[0:1, 0:1])
                    s["biasqT"] = pb.tile([128, 2], F32, tag="biasqT",
                                          name="biasqT")
                    for mi in range(2):
                        nc.vector.tensor_copy(s["biasqT"][:, mi:mi + 1],
                                              ptt[:, 2 * mi:2 * mi + 1])
                cl.append(c_biasq)

                def c_ncw(dk):
                    # ncW^T[d, q] = sum_e Wks[e, d] ncT[e, q]
                    if "ncWT" not in s:
                        s["ncWT"] = [None] * 4
                    pt = mm_ps.tile([128, 512], F32, tag="mm", name="mm")
                    for ne in range(4):
                        nc.tensor.matmul(
                            pt[:, 0:256], wksN[ne][:, dk * 128:(dk + 1) * 128],
                            s["ncTr"][ne], start=(ne == 0), stop=(ne == 3))
                    t = pb.tile([128, 256], BF16, tag=f"ncw{dk}",
                                name=f"ncw{dk}")
                    nc.vector.tensor_copy(t, pt[:, 0:256])
                    s["ncWT"][dk] = t
                for dk in range(4):
                    cl.append(lambda dk=dk: c_ncw(dk))

                def c_fin(mi, lg):
                    pt = mm_ps.tile([128, 512], F32, tag="mm", name="mm")
                    for nk in range(4):
                        nc.tensor.matmul(
                            pt, s["ncWT"][nk][:, mi * 128:(mi + 1) * 128],
                            s["kTT"][nk][:, lg * 512:(lg + 1) * 512],
                            start=(nk == 0), stop=(nk == 3))
                    th = thp.tile([128, 512], F32, tag="th", name="th")
                    nc.scalar.activation(th, pt, AF.Tanh,
                                         bias=s["biasqT"][:, mi:mi + 1],
                                         scale=ISQ_D)
                    th2 = thp.tile([128, 512], F32, tag="th2", name="th2")
                    nc.vector.tensor_scalar_mul(th2, th, CLIP)
                    nc.vector.copy_predicated(
                        th2, s["masku8"][:, lg * 512:(lg + 1) * 512],
                        fmin.to_broadcast([128, 512]))
                    nc.sync.dma_start(
                        out=out_d[bi, mi * 128:(mi + 1) * 128,
                                  lg * 512:(lg + 1) * 512],
                        in_=th2)
                for mi in range(2):
                    for lg in range(4):
                        cl.append(lambda mi=mi, lg=lg: c_fin(mi, lg))
                return cl

            # -------- attention unit machinery --------
            def emit_S(pos, t_, lc):
                s = S(pos)
                sp = st_ps.tile([128, 512], F32, tag="st", name="st")
                nc.tensor.matmul(
                    sp, s["kpTr"][t_][:, lc * 128:(lc + 1) * 128],
                    s["qpPad"][t_], start=True, stop=True)
                s[("sp", t_, lc)] = sp

            def emit_exp(pos, t_, lc):
                s = S(pos)
                et = etp.tile([128, 512], BF16, tag="et", name="et")
                nc.scalar.activation(
                    et, s.pop(("sp", t_, lc)), AF.Exp,
                    bias=s["mb"][:, lc:lc + 1], scale=ISQ_HD)
                s[("et", t_, lc)] = et

            def emit_U(pos, t_, lc):
                s = S(pos)
                if lc == 0:
                    s[("u", t_)] = u_ps.tile([128, 512], F32, tag="u", name="u")
                u = s[("u", t_)]
                et = s.pop(("et", t_, lc))
                vt = s["vpa"][lc]
                nc.tensor.matmul(
                    u[0:65, 0:256], vt[:, (2 * t_) * 65:(2 * t_) * 65 + 65],
                    et[:, 0:256], start=(lc == 0), stop=(lc == NLB - 1),
                    skip_group_check=True)
                # start=True on the even head cleared the whole bank; odd
                # head's first matmul relies on has_written=0 -> overwrite.
                nc.tensor.matmul(
                    u[0:65, 256:512],
                    vt[:, (2 * t_ + 1) * 65:(2 * t_ + 1) * 65 + 65],
                    et[:, 256:512], start=False, stop=(lc == NLB - 1),
                    skip_group_check=True)

            def emit_ctx(pos, t_):
                s = S(pos)
                u = s.pop(("u", t_))
                uf = smal.tile([128, 512], F32, tag="uf", name="uf")
                nc.vector.tensor_copy(uf[0:65, :], u[0:65, :])
                zr = smal.tile([1, 512], BF16, tag="zr", name="zr")
                nc.vector.tensor_copy(zr, uf[64:65, :])
                zb = mm_ps.tile([128, 512], F32, tag="mm", name="mm")
                nc.tensor.matmul(zb[0:64, :], onr[0:1, 0:64], zr,
                                 start=True, stop=True)
                rzt = smal.tile([64, 512], F32, tag="rzt", name="rzt")
                nc.vector.reciprocal_approx_fast(rzt, zb[0:64, :])
                if "ctx" not in s:
                    s["ctx"] = [None] * 4
                ct = pb.tile([64, 512], BF16, tag=f"ctx{t_}", name=f"ctx{t_}")
                nc.vector.tensor_mul(ct, uf[0:64, :], rzt)
                s["ctx"][t_] = ct

            def run_units(pos, fillers):
                # 2-deep S lookahead (3 st banks) + 1-deep Exp lookahead:
                # U(i) reads et(i) whose Exp was issued a full unit earlier.
                fill = deque(fillers)
                emit_S(pos, 0, 0)
                emit_exp(pos, 0, 0)
                for i in range(64):
                    t_, lc = divmod(i, 16)
                    if i + 1 < 64:
                        nt, nl = divmod(i + 1, 16)
                        emit_S(pos, nt, nl)
                        emit_exp(pos, nt, nl)
                    # fillers BEFORE U: PE chews projection work while ACT
                    # finishes Exp
                    n = -(-len(fill) // (64 - i))  # ceil
                    for _ in range(min(n, 4)):
                        if fill:
                            fill.popleft()()
                    emit_U(pos, t_, lc)
                    if lc == NLB - 1:
                        emit_ctx(pos, t_)
                while fill:
                    fill.popleft()()

            # ---------------- main schedule ----------------
            for clo in f_pre(0):
                clo()
            for pos in range(NB):
                fillers = []
                if pos > 0:
                    fillers += f_out(pos - 1)
                if pos + 1 < NB:
                    fillers += f_pre(pos + 1)
                run_units(pos, fillers)
                # free dead per-batch state
                if pos > 0:
                    st_.pop(pos - 1, None)
            for clo in f_out(NB - 1):
                clo()
    lowp.__exit__(None, None, None)
    nc.finalize()
    return nc


def kernel(**inputs):
    global LAST_RESULTS
    import os
    reps = int(os.environ.get("KERNEL_REPS", "1"))
    key = ("nc", reps)
    if key not in _CACHE:
        _CACHE[key] = _build(reps)
    nc = _CACHE[key]

    q = np.ascontiguousarray(np.asarray(inputs["q"], dtype=np.float32))
    k = np.ascontiguousarray(np.asarray(inputs["k"], dtype=np.float32))
    mask = np.ascontiguousarray(np.asarray(inputs["mask"]).astype(np.uint8))
    ws = {n: np.ascontiguousarray(np.asarray(inputs[n], dtype=np.float32))
          for n in W_NAMES}
    bs = {n: np.ascontiguousarray(np.asarray(inputs[n], dtype=np.float32))
          for n in ("bq", "bv", "bo", "bks")}

    in_maps = []
    for ci in range(NCORES):
        sl = slice(ci * BPC, (ci + 1) * BPC)
        im = {"q": q[sl], "k": k[sl], "mask": mask[sl]}
        im.update(ws)
        im.update(bs)
        in_maps.append(im)

    res = bass_utils.run_bass_kernel_spmd(
        nc, in_maps, core_ids=list(range(NCORES)), trace=TRACE)
    LAST_RESULTS = res
    out = np.concatenate([res.results[ci]["out"] for ci in range(NCORES)], axis=0)
    return out
